# revision 69
# baseline (speedup 1.0000x reference)
"""Bass/Tile TRN2 kernel for bilinear-score attention (score softmax + context).

reference:
    qW     = query @ W                      [B, Tq, Dk]
    weight = qW @ keys^T + mask[:, None, :] [B, Tq, Tk]
    score  = softmax(weight, axis=-1)
    ctx    = score @ values                 [B, Tq, Dv]
    returns (score, ctx)

Sharding: data-parallel over batch B=16 across 8 NeuronCores (2 batches/core).

Numerics: single-pass float32r matmuls for both big contractions. The PE runs
f32r at full rate (1 cycle/row) for outputs >= 256 wide, at ~tf32 operand
precision (~13 mantissa bits, measured dot-product relmax ~1.6e-4 over K=128).
The resulting logit noise (sigma ~8e-3 absolute on logits of std ~32) is
tolerable because softmax self-normalizes relative logit errors: score error
~= s*(1-s)*(dl_i - dl_top), so near-one-hot rows cancel almost entirely.
Walrus requires f32r matmul operands to be produced as f32r (rounded on
write): DMA loads into f32r tiles and ACT/DVE copies with f32r outputs both
qualify. The mask add stays exact fp32 during the PSUM drain. Phase 3
(score @ values) runs in fp16 with the score transposed on the PE through an
fp16 PSUM bitcast view. No DRAM scratch round trips anywhere.

Schedule per batch: load q row-blocks + W column-blocks -> PE-transpose qT ->
phase 1 with the kT transposes INTERLEAVED into its et steps (kt_sched, so
the kT psum drains overlap the matmul bursts instead of stalling the PE in a
dedicated block) -> software-pipelined phases 2+3 (step j: logits+softmax for
qt=j, score transpose for qt=j-2, ctx matmuls for qt=j-3). The softmax
normalization writes fp16 directly (s16_direct) and the score goes to DRAM
via an SWDGE cast-store, freeing the ACT engine and the SP hwdge queue.
Batch b+1's staging (q/k load issues + its qT PE transposes) rides inside
phase23(b) via pe_hooks (xbatch), and qT/kT live in SEPARATE pools
(split_xt) so the next batch's qT writes don't serialize against this
batch's kT reads -- together these remove the ~6 us inter-batch staging
bubble. values arrive as fp16 via SWDGE cast-loads; their tile_wait_until
floor steers the STATIC scheduler only (the timeline sim has no consumer for
bass_wait_until_ts), which is still enough to keep them out of the cold
staging DMA window.

Measured (cost-model timeline, device-verified numerics): 207.5 us/core
(baseline of this optimization round: 213.6 us; original 3-pass fp16 hi/lo:
429 us); score/ctx rel err 7.6e-3 vs fp32 reference (gate 2e-2). PE busy
194.3 us = 92.1%; remaining idle: ~4.3 us cold-start DMA latency, ~4.1 us
tail (last drain+DMA+final sems), ~3.5 us DMA-paced q staging.
Note: the two ctx accumulation halves must live in DIFFERENT PSUM tiles --
halves of one tile carry a false WAR dependency (vc1 matmuls wait on the
vc0 drain); vc1 borrows the free upper half of the score-transpose scratch
tile (or a second psA tile when sT_dma=1).

Tried and rejected (cost-model timeline, this round):
- sT_dma: score transpose on the DMA crossbar (InstDmaTransposeAnt, one call
  per [128,1024] fp16 row-block writing the [k,kt,q] layout directly; 64
  xbar tiles x 14ns). Saves 6.8 us of PE but loses 13+ us to serialized
  queue effects: the transpose's sem wait clears ~4 us after its input mul
  (sem batching), it head-blocks whichever hwdge queue carries it, and the
  extra 28.7 us of DMA_ENGINES occupancy starves the cx drain/DMA chain.
  220.3 us vs 207.5.
- cold_fuse: accumulating phase-1 et0/et1 inside the q staging loop with
  reordered W/k loads. Theory says p2(0) could start ~44 us; every variant
  measured 55-58 us (worse than the plain path's 53.3) -- the serialized
  DMA_ENGINES bandwidth means any load reorder delays q/k arrival, and the
  staging window stays DMA-bound regardless. 210.4-214.1 us.
- tail_chunks>1, cx_eng=act, mm_first, split_drain, split_first, deeper
  mm_off: all regress by 0.4-30 us (queue head-blocking or buffer churn).
- cold_swdge: cold q/k staging loads via SWDGE to parallelize descriptor
  generation past the shared-HWDGE 625ns serialization. 218-229 us — the
  slower Pool issue cadence (994+0.34/desc) and dge delays reorder the
  DMA_ENGINES grants and starve the transpose pipeline.

Known-broken paths (device runtime or walrus): DVE tensor_tensor_reduce
(INTERNAL error at run), fp16-identity transposes of 32-bit data and any
16/32-bit matmul operand mix (walrus NCC_IBIR034), gpsimd memset on f32r
tiles (NCC_IXCG864), per-block SBUF->SBUF dma_start_transpose for the score
(descriptor-generation bound: 625 ns/call swamps the 224 ns transfer).
"""

import os
import sys

import numpy as np

os.environ.setdefault("JAX_COMPILATION_CACHE_DIR", "/tmp/jax_comp_cache")

for _p in ("/opt/trn_rl_repo",):
    if _p not in sys.path and os.path.isdir(_p):
        sys.path.insert(0, _p)

import concourse.bass as bass  # noqa: E402
import concourse.tile as tile  # noqa: E402
from concourse import bacc, mybir  # noqa: E402
from concourse.bass import ds, ts  # noqa: E402
from concourse.bass_utils import run_bass_kernel_spmd  # noqa: E402

import json as _json

OPTS = {
    "ident_t": "f32r",   # identity dtype for fp32 PE transposes: f32r|f32|f16
    "v_floor_ms": 0.03,  # not_before floor for values SWDGE loads (batch 0)
    "v_floor2_ms": None, # batch-1 floor; None = v_floor_ms * 2
    "stage_bufs": 6,
    "soft_bufs": 2,
    "st_bufs": 3,
    "cx_bufs": 6,
    "tail_dve": 1,
    "s16_dve_last": 1,
    "ktail_dve": 0,
    "qtail_dve": 0,
    "cx_act": 0,
    "mm_off": 3,
    "split_drain": 0,
    "mm_first": 0,
    "last_cx_act": 0,
    "split_first": 0,
    # new knobs
    "sT_dma": 0,         # score transpose on the DMA xbar instead of the PE
    "s16_direct": 1,     # softmax mul writes fp16; score out via SWDGE cast-store
    "split_xt": 1,       # separate qT / kT pools (kills false cross-tensor serialization)
    "s16_bufs": 2,
    "exp_inplace": 1,    # Exp writes back into wsb (saves the expt tile)
    "cx_half": 1,        # cx drain tiles are per-half [P, 512]
    "sT_off": 1,         # pipeline offset for the score-transpose DMA emission
    "sT_eng": "sp",      # hwdge queue for the score-transpose DMA: sp|act
    "cx_eng": "sp",      # hwdge queue for the ctx output DMAs: sp|act
    "tail_chunks": 1,    # final ctx half drains/DMAs in this many chunks
    "kT_inter": 1,       # interleave keys transposes into phase-1 et steps
    "kt_sched": {5: [0, 1], 6: [2, 3], 7: [4, 5, 6, 7]},
    "xbatch": 1,         # stage batch b+1 (loads + qT) inside phase23(b)
    "xk_step": 2,        # phase23 step at which the next batch's k loads issue
    "cold_swdge": "",    # route cold-batch staging loads via SWDGE: ""|q|k|qk
    "v_pos": "early",    # values-load emission: early (pre-phase1) | late
    "v_dep_et": 4,       # late mode, cold batch: values wait for this et drain
    "v_dep_et_warm": 0,  # late mode, warm batches: values wait for this et drain
    "cold_fuse": 0,      # batch-0: fuse et0/et1 accumulation into q staging
    "q0_split": 4,       # split of the first q row-block load
    "mm_lag": 2,         # fused et0/et1 mm runs this many tiles behind qT
    "cold_order": [("q", 1), ("W", 0), ("W", 1),
                   ("q", 2), ("q", 3), ("q", 4), ("q", 5), ("q", 6), ("q", 7),
                   ("W", 2), ("k", 0), ("W", 3), ("k", 1), ("W", 4), ("k", 2),
                   ("W", 5), ("k", 3), ("k", 4), ("W", 6), ("k", 5), ("W", 7),
                   ("k", 6), ("k", 7)],
}
if os.environ.get("K_OPTS"):
    OPTS.update(_json.loads(os.environ["K_OPTS"]))
    if isinstance(OPTS.get("kt_sched"), dict):
        OPTS["kt_sched"] = {int(k): v for k, v in OPTS["kt_sched"].items()}

P = 128
T = 1024
NT = T // P  # 8
NB = 2       # batches per core
NCORES = 8
F32 = mybir.dt.float32
F16 = mybir.dt.float16
F32R = mybir.dt.float32r
AX = mybir.AxisListType
AOP = mybir.AluOpType
AF = mybir.ActivationFunctionType


def _issue_loads(nc, pools, src, tdt, trange=None, eng=None):
    """Issue row-block DMA loads for a [1024, 1024] DRAM tensor. eng=gpsimd
    routes through SWDGE, whose descriptor generation runs on the Pool
    engine in parallel with the serialized shared-HWDGE issue path."""
    stage = pools["stage"]
    eng = eng or nc.sync
    tiles = []
    for t_ in trange if trange is not None else range(NT):
        xf = stage.tile([P, T], tdt, tag="ldf32", bufs=OPTS["stage_bufs"])
        src_ap = src[ts(t_, P), :]
        if tdt != F32:
            src_ap = src_ap.bitcast(tdt)
        if OPTS["split_first"] and t_ == 0 and trange is None:
            eng.dma_start(xf[:, ds(0, 512)], src_ap[:, ds(0, 512)])
            eng.dma_start(xf[:, ds(512, 512)], src_ap[:, ds(512, 512)])
        else:
            eng.dma_start(xf[:], src_ap)
        tiles.append(xf)
    return tiles


def _cold_stage(nc, pools, q_d0, k_d0, w_d, tdt):
    """Batch-0 cold start: q row-blocks, W column-blocks and keys row-blocks
    issued in an order matched to the serialized DMA bandwidth — q0 (split
    for an earlier first transpose), W0/W1 (needed by the fused et0/et1
    accumulation), the rest of q, then W2..7 interleaved with keys so each
    lands just before its consumer."""
    stage = pools["stage"]
    qtiles = []
    xf = stage.tile([P, T], tdt, tag="ldf32", bufs=OPTS["stage_bufs"])
    src = q_d0[ts(0, P), :]
    if tdt != F32:
        src = src.bitcast(tdt)
    ns = OPTS["q0_split"]
    cw = T // ns
    for c in range(ns):
        nc.sync.dma_start(xf[:, ds(c * cw, cw)], src[:, ds(c * cw, cw)])
    qtiles.append(xf)
    W_t = [None] * NT
    ktiles = []
    for kind, i in OPTS["cold_order"]:
        if kind == "W":
            W_t[i] = _issue_w_load(nc, pools, w_d, i)
        elif kind == "q":
            qtiles += _issue_loads(nc, pools, q_d0, tdt, trange=[i])
        else:
            ktiles += _issue_loads(nc, pools, k_d0, tdt, trange=[i])
    return qtiles, W_t, ktiles


def _fused_qT_et01(nc, pools, qtiles, W_t, qdrain):
    """Transpose the q row-blocks and accumulate phase-1 et0/et1 in the same
    loop, one dt behind each transpose (so the mm never waits on the psum
    drain), turning the DMA-bound staging window into PE work and keeping
    the tensor-engine clock ramped."""
    psA = pools["psA"]
    qT = pools["xt"].tile([P, NT, T], F32R, tag="xT")
    ps_e0 = psA.tile([P, T], F32, tag="psA")
    ps_e1 = psA.tile([P, T], F32, tag="psA")
    ps_e = [ps_e0, ps_e1]

    def mm(d):
        for eti in range(2):
            for qc in range(2):
                nc.tensor.matmul(
                    ps_e[eti][:, ds(qc * 512, 512)], W_t[eti][:, d, :],
                    qT[:, d, ds(qc * 512, 512)],
                    start=(d == 0), stop=(d == NT - 1),
                )

    lag = OPTS["mm_lag"]
    for t in range(NT):
        _transpose_from(nc, pools, [qtiles[t]], qdrain, xT=qT, t0=t)
        if t >= lag:
            mm(t - lag)
    for d in range(NT - lag, NT):
        mm(d)
    return qT, ps_e


def _transpose_from(nc, pools, tiles, drain_eng, xT=None, t0=0, pool="xt"):
    """xT[d, o, blk] = src^T from pre-loaded row-block stage tiles."""
    psB = pools["psB"]
    idt = pools["identT"]
    tdt = idt.dtype
    if xT is None:
        xT = pools[pool].tile([P, NT, T], F32R, tag="xT")
    for i, xf in enumerate(tiles):
        t_ = t0 + i
        pst = psB.tile([P, T], F32, tag="psB")
        out_v = pst[:] if tdt == F32 else pst[:].bitcast(tdt)
        for b_ in range(NT):
            nc.tensor.transpose(out_v[:, ts(b_, P)], xf[:, ts(b_, P)], idt[:])
        view = pst[:].rearrange("p (o q) -> p o q", q=P)
        if OPTS["split_drain"]:
            # halves drain concurrently on ACT+DVE so the PSUM buffer frees
            # in ~660ns, keeping up with the 640ns transpose cadence
            H = NT // 2
            nc.scalar.copy(xT[:, ds(0, H), ts(t_, P)], view[:, ds(0, H), :])
            nc.vector.tensor_copy(xT[:, ds(H, H), ts(t_, P)], view[:, ds(H, H), :])
        else:
            drain_eng(xT[:, :, ts(t_, P)], view)
    return xT


def _issue_w_load(nc, pools, w_d, et):
    """One W column-block as lhsT layout [d%128, d//128, e]. Separate tiles
    per 128-col block keep the dependency regions disjoint (a single tile's
    strided column writes get bounding-box-merged, making phase 1 et=0 wait
    on several W DMAs instead of one)."""
    W_t = pools["wres"].tile([P, NT, P], F32R, tag=f"W{et}")
    nc.sync.dma_start(
        W_t[:],
        w_d[:, ts(et, P)].rearrange("(o p) e -> p o e", p=P).bitcast(F32R),
    )
    return W_t


def _issue_w_loads(nc, pools, w_d):
    return [_issue_w_load(nc, pools, w_d, et) for et in range(NT)]


def _phase1(nc, pools, W_t, qT, kt_work=None, pre=None):
    """qWT[e, et, q] = W^T @ query^T, single-pass f32r.

    kt_work: optional {et_step: [thunk, ...]} of PE-side staging work (keys
    row-block transposes) interleaved after the given et steps, so the kT
    psum drains overlap the phase-1 matmul bursts instead of stalling the PE
    in a dedicated transpose block afterwards."""
    qwt_pool, psA = pools["qwt"], pools["psA"]
    qWT = qwt_pool.tile([P, NT, T], F32R, tag="qWT")
    et0 = 0
    drains = []
    if pre is not None:
        # et0/et1 already accumulated during the fused staging loop
        for eti, ps in enumerate(pre):
            drains.append(nc.vector.tensor_copy(qWT[:, eti, :], ps[:]))
        et0 = len(pre)
    for et in range(et0, NT):
        ps = psA.tile([P, T], F32, tag="psA")
        for dt_ in range(NT):
            lw = W_t[et][:, dt_, :]
            for qc in range(2):
                nc.tensor.matmul(
                    ps[:, ds(qc * 512, 512)], lw,
                    qT[:, dt_, ds(qc * 512, 512)],
                    start=(dt_ == 0), stop=(dt_ == NT - 1),
                )
        drains.append(nc.vector.tensor_copy(qWT[:, et, :], ps[:]))
        if kt_work:
            for thunk in kt_work.get(et, ()):
                thunk()
    return qWT, drains


def _p2_step(nc, pools, b, s_d, qWT, kT, mrep, qt_):
    """One qt row-block of phase 2: logits matmuls (chunk-outer so chunk 0
    starts before the last kT drains land), then the softmax chain."""
    soft, psA = pools["soft"], pools["psA"]
    ps2 = psA.tile([P, T], F32, tag="psA")
    for kc in range(2):
        for et in range(NT):
            nc.tensor.matmul(
                ps2[:, ds(kc * 512, 512)], qWT[:, et, ts(qt_, P)],
                kT[:, et, ds(kc * 512, 512)],
                start=(et == 0), stop=(et == NT - 1),
            )
    # drain + mask add fused: wsb = ps2 + mask (fp32, exact)
    wsb = soft.tile([P, T], F32, tag="wsb")
    nc.vector.tensor_tensor(wsb[:], ps2[:], mrep[:], AOP.add)
    negmax = soft.tile([P, 1], F32, tag="negmax")
    nc.vector.tensor_reduce(negmax[:], wsb[:], axis=AX.X, op=AOP.max, negate=True)
    sumexp = soft.tile([P, 1], F32, tag="sumexp")
    if OPTS["exp_inplace"]:
        expt = wsb
    else:
        expt = soft.tile([P, T], F32, tag="expt")
    nc.scalar.activation(
        expt[:], wsb[:], AF.Exp, bias=negmax[:], scale=1.0, accum_out=sumexp[:]
    )
    recip = soft.tile([P, 1], F32, tag="recip")
    nc.vector.reciprocal(recip[:], sumexp[:])
    if OPTS["s16_direct"]:
        # normalize straight into fp16; score goes to DRAM via SWDGE
        # cast-store (f16 -> f32). abs err <= 5e-4 on scores in [0,1].
        # distinct per-slot tags = distinct SBUF tensors, so the region
        # tracker can't bounding-box-merge adjacent slots' writes
        s16 = pools["sc"].tile(
            [P, T], F16, tag=f"s16_{qt_ % OPTS['s16_bufs']}", bufs=1
        )
        nc.vector.tensor_scalar_mul(s16[:], expt[:], recip[:])
        nc.gpsimd.dma_start(s_d[b, ts(qt_, P), :], s16[:])
        return s16
    nc.vector.tensor_scalar_mul(expt[:], expt[:], recip[:])
    nc.sync.dma_start(s_d[b, ts(qt_, P), :], expt[:])
    s16 = pools["sc"].tile([P, T], F16, tag=f"s16_{qt_}")
    if qt_ >= NT - OPTS["s16_dve_last"]:
        nc.vector.tensor_copy(s16[:], expt[:])
    else:
        nc.scalar.copy(s16[:], expt[:])
    return s16


def _p3_transpose_dma(nc, pools, s16, qt_):
    """Score transpose on the DMA crossbar: one InstDmaTransposeAnt per
    [128, 1024] fp16 row-block, writing the [k, kt, q] SBUF layout directly
    (64 xbar tiles x 14 ns ~= 0.9 us on the shared DMA engines; zero PE/ACT).
    Emitted one pipeline step late (sT_off) so the issuing queue never
    head-blocks waiting for the softmax chain."""
    sT16 = pools["st"].tile(
        [P, NT, P], F16, tag=f"sT16_{qt_ % OPTS['st_bufs']}", bufs=1
    )
    eng = nc.scalar if OPTS["sT_eng"] == "act" else nc.sync
    eng.dma_start(sT16[:], s16[:], transpose=True)
    return sT16


def _p3_transpose(nc, pools, s16, dve_drain=False):
    """Transpose one qt row-block of fp16 scores on the PE via an fp16 PSUM
    bitcast view; drain to a [k, kt, q] SBUF tile. Tail transposes drain on
    DVE so the ACT queue is clear when the next batch's staging drains start."""
    psB = pools["psB"]
    ident16 = pools["ident16"]
    ps = psB.tile([P, T], F32, tag="psB")
    view = ps[:, ds(0, 512)].bitcast(F16)
    for kt_ in range(NT):
        nc.tensor.transpose(view[:, ts(kt_, P)], s16[:, ts(kt_, P)], ident16[:])
    sT16 = pools["st"].tile([P, NT, P], F16, tag="sT16", bufs=OPTS["st_bufs"])
    rview = view.rearrange("p (o q) -> p o q", q=P)
    if dve_drain:
        nc.vector.tensor_copy(sT16[:], rview)
    else:
        nc.scalar.copy(sT16[:], rview)
    return sT16, ps


def _p3_mm(nc, pools, b, c_d, sT16, ps_tr, vals, qt_, last=False):
    """ctx[qt block] = score^T^T @ values, fp16. The two 512-col halves
    accumulate into DIFFERENT PSUM TILES (vc1 borrows the free upper half of
    the transpose scratch) so vc1's matmuls carry no false dependency on the
    vc0 drain; ctx DMAs out per half."""
    cx_pool, psA = pools["cx"], pools["psA"]
    ps3 = psA.tile([P, T], F32, tag="psA")
    if ps_tr is None:
        # no PE-transpose scratch to borrow: take a second psA tile so the
        # vc1 matmuls carry no false WAR on the vc0 drain
        ps_tr = psA.tile([P, T], F32, tag="psA")
    halves = (ps3[:, ds(0, 512)], ps_tr[:, ds(512, 512)])
    if not OPTS["cx_half"]:
        cxt_full = cx_pool.tile([P, T], F32, tag="cx")
    for vc in range(2):
        half = halves[vc]
        for kt_ in range(NT):
            nc.tensor.matmul(
                half, sT16[:, kt_, :], vals[:, kt_, ds(vc * 512, 512)],
                start=(kt_ == 0), stop=(kt_ == NT - 1),
            )
        if last and vc == 1 and OPTS["tail_chunks"] > 1:
            # final drain+DMA in fine chunks on alternating engines so the
            # post-last-matmul critical path is one small chunk, not 512 cols
            ncH = OPTS["tail_chunks"]
            w = 512 // ncH
            cxt = cx_pool.tile([P, 512], F32, tag="cx", bufs=OPTS["cx_bufs"])
            for c in range(ncH):
                if c % 2:
                    nc.scalar.copy(cxt[:, ds(c * w, w)], half[:, ds(c * w, w)])
                else:
                    nc.vector.tensor_copy(cxt[:, ds(c * w, w)],
                                          half[:, ds(c * w, w)])
                nc.sync.dma_start(
                    c_d[b, ts(qt_, P), ds(512 + c * w, w)],
                    cxt[:, ds(c * w, w)])
            continue
        if OPTS["cx_half"]:
            cxt = cx_pool.tile([P, 512], F32, tag="cx", bufs=OPTS["cx_bufs"])
            cview = cxt[:]
        else:
            cview = cxt_full[:, ds(vc * 512, 512)]
        if OPTS["cx_act"] > vc or (last and vc == 1 and OPTS["last_cx_act"]):
            nc.scalar.copy(cview, half)
        else:
            nc.vector.tensor_copy(cview, half)
        cx_dma_eng = nc.scalar if OPTS["cx_eng"] == "act" else nc.sync
        cx_dma_eng.dma_start(c_d[b, ts(qt_, P), ds(vc * 512, 512)], cview)


def _phase23(nc, pools, b, s_d, c_d, qWT, kT, mrep, vals, pe_hooks=None):
    """Software-pipelined phases 2+3: step j runs p2(qt=j), the score
    transpose for qt=j-1, and the ctx matmuls for qt=j-2, keeping the PE fed
    while the ACT/DVE drains of earlier tiles complete.

    pe_hooks: optional {step: [thunk, ...]} of extra work (the next batch's
    staging load issues / qT transposes) emitted right after the p2 step."""
    s16s, sT16s = {}, {}
    MO = OPTS["mm_off"]
    TO = OPTS["sT_off"]
    for j in range(NT + MO):
        if j < NT:
            s16s[j] = _p2_step(nc, pools, b, s_d, qWT, kT, mrep, j)
        if pe_hooks:
            for thunk in pe_hooks.get(j, ()):
                thunk()
        if OPTS["sT_dma"] and TO <= j < NT + TO:
            sT16s[j - TO] = (_p3_transpose_dma(nc, pools, s16s[j - TO], j - TO), None)
        if OPTS["mm_first"] and j >= MO:
            sT16, ps_tr = sT16s[j - MO]
            _p3_mm(nc, pools, b, c_d, sT16, ps_tr, vals, j - MO)
        if not OPTS["sT_dma"] and 2 <= j <= NT + 1:
            sT16s[j - 2] = _p3_transpose(nc, pools, s16s[j - 2],
                                         dve_drain=(j >= NT + 1 - OPTS["tail_dve"]))
        if not OPTS["mm_first"] and j >= MO:
            sT16, ps_tr = sT16s[j - MO]
            _p3_mm(nc, pools, b, c_d, sT16, ps_tr, vals, j - MO,
                   last=(b == NB - 1 and j - MO == NT - 1))


def _load_values(nc, pools, b, v_d, after=None, not_before_ms=0.0):
    """SWDGE cast-load: fp32 DRAM -> fp16 SBUF, no compute engine involved.
    not_before_ms steers the static scheduler's placement; `after` (an
    earlier instruction) optionally gates the first load via a real
    dependency edge (the Pool queue is in-order, so one edge holds all
    eight)."""
    vals = pools["val"].tile([P, NT, T], F16, tag="vals")
    tc = pools["tc"]
    with tc.tile_wait_until(not_before_ms):
        for kt_ in range(NT):
            dma = nc.gpsimd.dma_start(vals[:, kt_, :], v_d[b, ts(kt_, P), :])
            if kt_ == 0 and after is not None:
                bass._add_dep_helper(dma.ins, after.ins, sync=True,
                                     reason="defer values past staging crunch")
    return vals


def _stage_mask(nc, pools, b, m_d):
    stage, small = pools["stage"], pools["small"]
    mf = stage.tile([P, T], F32, tag="ldmask", bufs=1)
    nc.gpsimd.dma_start(mf[:1, :], m_d[b : b + 1, :])
    mrep = small.tile([P, T], F32, tag="mrep")
    nc.gpsimd.partition_broadcast(mrep[:], mf[:1, :])
    return mrep


def build_nc(reps=1):
    nc = bacc.Bacc("TRN2", target_bir_lowering=False, debug=False, num_devices=NCORES)
    q_d = nc.dram_tensor("query", [NB, T, T], F32, kind="ExternalInput")
    k_d = nc.dram_tensor("keys", [NB, T, T], F32, kind="ExternalInput")
    v_d = nc.dram_tensor("values", [NB, T, T], F32, kind="ExternalInput")
    w_d = nc.dram_tensor("W", [T, T], F32, kind="ExternalInput")
    m_d = nc.dram_tensor("mask", [NB, T], F32, kind="ExternalInput")
    s_d = nc.dram_tensor("score", [NB, T, T], F32, kind="ExternalOutput")
    c_d = nc.dram_tensor("ctx", [NB, T, T], F32, kind="ExternalOutput")

    with tile.TileContext(nc) as tc:
        with (
            tc.tile_pool(name="stage", bufs=2) as stage,
            tc.tile_pool(name="wres", bufs=1) as wres,
            tc.tile_pool(name="xt", bufs=1) as xt_pool,
            tc.tile_pool(name="kt", bufs=1) as kt_pool,
            tc.tile_pool(name="qwt", bufs=1) as qwt_pool,
            tc.tile_pool(name="val", bufs=1) as val_pool,
            tc.tile_pool(name="sc", bufs=1) as sc_pool,
            tc.tile_pool(name="soft", bufs=OPTS["soft_bufs"]) as soft,
            tc.tile_pool(name="st", bufs=2) as st_pool,
            tc.tile_pool(name="cx", bufs=OPTS["cx_bufs"]) as cx_pool,
            tc.tile_pool(name="small", bufs=1) as small,
            tc.tile_pool(name="ones", bufs=1) as ones_pool,
        ):
            with (
                tc.tile_pool(name="psA", bufs=2, space="PSUM") as psA,
                tc.tile_pool(name="psB", bufs=2, space="PSUM") as psB,
            ):
                pools = {
                    "tc": tc, "wres": wres,
                    "stage": stage, "xt": xt_pool, "kt": kt_pool,
                    "qwt": qwt_pool,
                    "val": val_pool, "soft": soft, "st": st_pool, "sc": sc_pool,
                    "cx": cx_pool, "small": small,
                    "psA": psA, "psB": psB,
                }
                ident16 = ones_pool.tile([P, P], F16, tag="ident16")
                from concourse.masks import make_identity
                make_identity(nc, ident16[:])
                pools["ident16"] = ident16
                identF = ones_pool.tile([P, P], F32, tag="identF")
                make_identity(nc, identF[:])
                if OPTS["ident_t"] == "f32r":
                    identR = ones_pool.tile([P, P], F32R, tag="identR")
                    nc.scalar.copy(identR[:], identF[:])
                    pools["identT"] = identR
                else:
                    pools["identT"] = identF

                tdt = pools["identT"].dtype

                def mk_qdrain():
                    return lambda d, v, _i=iter(range(NT * NT)): (
                        nc.vector.tensor_copy(d, v)
                        if next(_i) >= NT - OPTS["qtail_dve"]
                        else nc.scalar.copy(d, v)
                    )

                kdrain = lambda d, v: nc.scalar.copy(d, v)
                kt_pool_name = "kt" if OPTS["split_xt"] else "xt"
                KSCHED = OPTS["kt_sched"]

                for _rep in range(reps):
                    prepared = {}
                    for b in range(NB):
                        pre_ps = None
                        if b in prepared:
                            qT, ktiles = prepared.pop(b)
                        elif OPTS["cold_fuse"] and _rep == 0 and b == 0:
                            qtiles, W_t, ktiles = _cold_stage(
                                nc, pools, q_d[b], k_d[b], w_d, tdt)
                            qT, pre_ps = _fused_qT_et01(
                                nc, pools, qtiles, W_t, mk_qdrain())
                        else:
                            cs = OPTS["cold_swdge"]
                            if cs == "q0":
                                qtiles = _issue_loads(
                                    nc, pools, q_d[b], tdt, trange=[0],
                                    eng=nc.gpsimd)
                                qtiles += _issue_loads(
                                    nc, pools, q_d[b], tdt,
                                    trange=range(1, NT))
                            else:
                                qtiles = _issue_loads(
                                    nc, pools, q_d[b], tdt,
                                    eng=nc.gpsimd if "q" in cs else None)
                            if _rep == 0 and b == 0:
                                W_t = _issue_w_loads(nc, pools, w_d)
                            qT = _transpose_from(nc, pools, qtiles, mk_qdrain())
                            ktiles = _issue_loads(
                                nc, pools, k_d[b], tdt,
                                eng=nc.gpsimd if "k" in cs else None)
                        mrep = _stage_mask(nc, pools, b, m_d)
                        vals = None
                        if OPTS["v_pos"] == "early":
                            nb_ms = (OPTS["v_floor_ms"] if b == 0
                                     else (OPTS["v_floor2_ms"]
                                           or OPTS["v_floor_ms"] * (b + 1)))
                            vals = _load_values(
                                nc, pools, b, v_d, not_before_ms=nb_ms,
                            )
                        if OPTS["kT_inter"]:
                            # keys transposes ride inside the phase-1 et
                            # steps; their psum drains overlap the matmuls
                            kT = pools[kt_pool_name].tile(
                                [P, NT, T], F32R, tag="xT"
                            )
                            kt_work = {
                                et: [
                                    (lambda i=i: _transpose_from(
                                        nc, pools, [ktiles[i]], kdrain,
                                        xT=kT, t0=i))
                                    for i in idxs
                                ]
                                for et, idxs in KSCHED.items()
                            }
                            qWT, p1dr = _phase1(nc, pools, W_t, qT, kt_work,
                                                pre=pre_ps)
                        else:
                            qWT, p1dr = _phase1(nc, pools, W_t, qT, pre=pre_ps)
                            kT = _transpose_from(
                                nc, pools, ktiles, kdrain, pool=kt_pool_name,
                            )
                        if vals is None:
                            vdep = (OPTS["v_dep_et"] if b == 0
                                    else OPTS["v_dep_et_warm"])
                            vals = _load_values(
                                nc, pools, b, v_d,
                                after=p1dr[vdep] if vdep is not None else None,
                            )
                        hooks = {}
                        if OPTS["xbatch"] and b + 1 < NB:
                            nb = b + 1
                            nqtiles_box = []
                            nqT = pools["xt"].tile([P, NT, T], F32R, tag="xT")
                            nqdrain = mk_qdrain()

                            def issue_next_q(_nb=nb, _box=nqtiles_box):
                                _box.extend(_issue_loads(nc, pools, q_d[_nb], tdt))

                            hooks.setdefault(0, []).append(issue_next_q)
                            for i in range(NT):
                                hooks.setdefault(i + 1, []).append(
                                    lambda i=i, _box=nqtiles_box: _transpose_from(
                                        nc, pools, [_box[i]], nqdrain,
                                        xT=nqT, t0=i)
                                )
                            nktiles_box = []

                            def issue_next_k(_nb=nb, _box=nktiles_box):
                                _box.extend(_issue_loads(nc, pools, k_d[_nb], tdt))

                            hooks.setdefault(OPTS["xk_step"], []).append(issue_next_k)
                            prepared[nb] = (nqT, nktiles_box)
                        _phase23(nc, pools, b, s_d, c_d, qWT, kT, mrep, vals,
                                 pe_hooks=hooks or None)

    nc.compile()
    return nc


_nc = None


def _get_nc():
    global _nc
    if _nc is None:
        _nc = build_nc()
    return _nc


def make_in_maps(query, keys, values, W, mask):
    query = np.ascontiguousarray(np.asarray(query, dtype=np.float32))
    keys = np.ascontiguousarray(np.asarray(keys, dtype=np.float32))
    values = np.ascontiguousarray(np.asarray(values, dtype=np.float32))
    W = np.ascontiguousarray(np.asarray(W, dtype=np.float32))
    mask = np.ascontiguousarray(np.asarray(mask, dtype=np.float32))
    in_maps = []
    for c in range(NCORES):
        sl = slice(c * NB, (c + 1) * NB)
        in_maps.append(
            {
                "query": query[sl],
                "keys": keys[sl],
                "values": values[sl],
                "W": W,
                "mask": mask[sl],
            }
        )
    return in_maps


def kernel(query, keys, values, W, mask):
    nc = _get_nc()
    in_maps = make_in_maps(query, keys, values, W, mask)
    res = run_bass_kernel_spmd(nc, in_maps, core_ids=list(range(NCORES)))
    score = np.concatenate([res.results[c]["score"] for c in range(NCORES)], axis=0)
    ctx = np.concatenate([res.results[c]["ctx"] for c in range(NCORES)], axis=0)
    return score, ctx



# revision 77
# speedup vs baseline: 1.0000x; 1.0000x over previous
"""Bass/Tile TRN2 kernel for bilinear-score attention (score softmax + context).

reference:
    qW     = query @ W                      [B, Tq, Dk]
    weight = qW @ keys^T + mask[:, None, :] [B, Tq, Tk]
    score  = softmax(weight, axis=-1)
    ctx    = score @ values                 [B, Tq, Dv]
    returns (score, ctx)

Sharding: data-parallel over batch B=16 across 8 NeuronCores (2 batches/core).

Numerics: single-pass float32r matmuls for both big contractions. The PE runs
f32r at full rate (1 cycle/row) for outputs >= 256 wide, at ~tf32 operand
precision (~13 mantissa bits, measured dot-product relmax ~1.6e-4 over K=128).
The resulting logit noise (sigma ~8e-3 absolute on logits of std ~32) is
tolerable because softmax self-normalizes relative logit errors: score error
~= s*(1-s)*(dl_i - dl_top), so near-one-hot rows cancel almost entirely.
Walrus requires f32r matmul operands to be produced as f32r (rounded on
write): DMA loads into f32r tiles and ACT/DVE copies with f32r outputs both
qualify. The mask add stays exact fp32 during the PSUM drain. Phase 3
(score @ values) runs in fp16 with the score transposed on the PE through an
fp16 PSUM bitcast view. No DRAM scratch round trips anywhere.

Schedule per batch: load q row-blocks + W column-blocks -> PE-transpose qT ->
phase 1 with the kT transposes INTERLEAVED into its et steps (kt_sched, so
the kT psum drains overlap the matmul bursts instead of stalling the PE in a
dedicated block) -> software-pipelined phases 2+3 (step j: logits+softmax for
qt=j, score transpose for qt=j-2, ctx matmuls for qt=j-3). The softmax
normalization writes fp16 directly (s16_direct) and the score goes to DRAM
via an SWDGE cast-store, freeing the ACT engine and the SP hwdge queue.
Batch b+1's staging (q/k load issues + its qT PE transposes) rides inside
phase23(b) via pe_hooks (xbatch), and qT/kT live in SEPARATE pools
(split_xt) so the next batch's qT writes don't serialize against this
batch's kT reads -- together these remove the ~6 us inter-batch staging
bubble. values arrive as fp16 via SWDGE cast-loads; their tile_wait_until
floor steers the STATIC scheduler only (the timeline sim has no consumer for
bass_wait_until_ts), which is still enough to keep them out of the cold
staging DMA window.

PE p-state note: the clock ramp (0.65 -> 1.2 -> 2.4 GHz after 3 us of
continuous execution) resets on idle gaps, so the cold-staging transposes
run at the low state inside DMA shadows. warm_n0/warm_n emit idempotent
re-transposes (identity into the block-0 psum region before data lands, a
repeat of the last block after) to hold the clock up through the DMA waits;
worth ~10 ns end-to-end here but harmless, kept for robustness.

Measured (cost-model timeline, device-verified numerics): 207.5 us/core
(baseline of this optimization round: 213.6 us; original 3-pass fp16 hi/lo:
429 us); score/ctx rel err 7.6e-3 vs fp32 reference (gate 2e-2). PE busy
194.3 us = 92.1%; remaining idle: ~4.3 us cold-start DMA latency, ~4.1 us
tail (last drain+DMA+final sems), ~3.5 us DMA-paced q staging.
Note: the two ctx accumulation halves must live in DIFFERENT PSUM tiles --
halves of one tile carry a false WAR dependency (vc1 matmuls wait on the
vc0 drain); vc1 borrows the free upper half of the score-transpose scratch
tile (or a second psA tile when sT_dma=1).

Tried and rejected (cost-model timeline, this round):
- sT_dma: score transpose on the DMA crossbar (InstDmaTransposeAnt, one call
  per [128,1024] fp16 row-block writing the [k,kt,q] layout directly; 64
  xbar tiles x 14ns). Saves 6.8 us of PE but loses 13+ us to serialized
  queue effects: the transpose's sem wait clears ~4 us after its input mul
  (sem batching), it head-blocks whichever hwdge queue carries it, and the
  extra 28.7 us of DMA_ENGINES occupancy starves the cx drain/DMA chain.
  220.3 us vs 207.5.
- cold_fuse: accumulating phase-1 et0/et1 inside the q staging loop with
  reordered W/k loads. Theory says p2(0) could start ~44 us; every variant
  measured 55-58 us (worse than the plain path's 53.3) -- the serialized
  DMA_ENGINES bandwidth means any load reorder delays q/k arrival, and the
  staging window stays DMA-bound regardless. 210.4-214.1 us.
- tail_chunks>1, cx_eng=act, mm_first, split_drain, split_first, deeper
  mm_off: all regress by 0.4-30 us (queue head-blocking or buffer churn).
- cold_swdge: cold q/k staging loads via SWDGE to parallelize descriptor
  generation past the shared-HWDGE 625ns serialization. 218-229 us — the
  slower Pool issue cadence (994+0.34/desc) and dge delays reorder the
  DMA_ENGINES grants and starve the transpose pipeline.

Known-broken paths (device runtime or walrus): DVE tensor_tensor_reduce
(INTERNAL error at run), fp16-identity transposes of 32-bit data and any
16/32-bit matmul operand mix (walrus NCC_IBIR034), gpsimd memset on f32r
tiles (NCC_IXCG864), per-block SBUF->SBUF dma_start_transpose for the score
(descriptor-generation bound: 625 ns/call swamps the 224 ns transfer).
"""

import os
import sys

import numpy as np

os.environ.setdefault("JAX_COMPILATION_CACHE_DIR", "/tmp/jax_comp_cache")

for _p in ("/opt/trn_rl_repo",):
    if _p not in sys.path and os.path.isdir(_p):
        sys.path.insert(0, _p)

import concourse.bass as bass  # noqa: E402
import concourse.tile as tile  # noqa: E402
from concourse import bacc, mybir  # noqa: E402
from concourse.bass import ds, ts  # noqa: E402
from concourse.bass_utils import run_bass_kernel_spmd  # noqa: E402

import json as _json

OPTS = {
    "ident_t": "f32r",   # identity dtype for fp32 PE transposes: f32r|f32|f16
    "v_floor_ms": 0.03,  # not_before floor for values SWDGE loads (batch 0)
    "v_floor2_ms": None, # batch-1 floor; None = v_floor_ms * 2
    "stage_bufs": 6,
    "soft_bufs": 2,
    "st_bufs": 3,
    "cx_bufs": 6,
    "tail_dve": 1,
    "s16_dve_last": 1,
    "ktail_dve": 0,
    "qtail_dve": 0,
    "cx_act": 0,
    "mm_off": 3,
    "split_drain": 0,
    "mm_first": 0,
    "last_cx_act": 0,
    "split_first": 0,
    # new knobs
    "sT_dma": 0,         # score transpose on the DMA xbar instead of the PE
    "s16_direct": 1,     # softmax mul writes fp16; score out via SWDGE cast-store
    "split_xt": 1,       # separate qT / kT pools (kills false cross-tensor serialization)
    "s16_bufs": 2,
    "exp_inplace": 1,    # Exp writes back into wsb (saves the expt tile)
    "cx_half": 1,        # cx drain tiles are per-half [P, 512]
    "sT_off": 1,         # pipeline offset for the score-transpose DMA emission
    "sT_eng": "sp",      # hwdge queue for the score-transpose DMA: sp|act
    "cx_eng": "sp",      # hwdge queue for the ctx output DMAs: sp|act
    "tail_chunks": 1,    # final ctx half drains/DMAs in this many chunks
    "kT_inter": 1,       # interleave keys transposes into phase-1 et steps
    "kt_sched": {5: [0, 1], 6: [2, 3], 7: [4, 5, 6, 7]},
    "xbatch": 1,         # stage batch b+1 (loads + qT) inside phase23(b)
    "xk_step": 2,        # phase23 step at which the next batch's k loads issue
    "cold_swdge": "",    # route cold-batch staging loads via SWDGE: ""|q|k|qk
    "warm_n0": 10,       # PE-warming dummy transposes before the first q tile
    "warm_n": 6,         # PE-warming dummies between cold staging transposes
    "v_pos": "early",    # values-load emission: early (pre-phase1) | late
    "v_dep_et": 4,       # late mode, cold batch: values wait for this et drain
    "v_dep_et_warm": 0,  # late mode, warm batches: values wait for this et drain
    "cold_fuse": 0,      # batch-0: fuse et0/et1 accumulation into q staging
    "q0_split": 4,       # split of the first q row-block load
    "mm_lag": 2,         # fused et0/et1 mm runs this many tiles behind qT
    "cold_order": [("q", 1), ("W", 0), ("W", 1),
                   ("q", 2), ("q", 3), ("q", 4), ("q", 5), ("q", 6), ("q", 7),
                   ("W", 2), ("k", 0), ("W", 3), ("k", 1), ("W", 4), ("k", 2),
                   ("W", 5), ("k", 3), ("k", 4), ("W", 6), ("k", 5), ("W", 7),
                   ("k", 6), ("k", 7)],
}
if os.environ.get("K_OPTS"):
    OPTS.update(_json.loads(os.environ["K_OPTS"]))
    if isinstance(OPTS.get("kt_sched"), dict):
        OPTS["kt_sched"] = {int(k): v for k, v in OPTS["kt_sched"].items()}

P = 128
T = 1024
NT = T // P  # 8
NB = 2       # batches per core
NCORES = 8
F32 = mybir.dt.float32
F16 = mybir.dt.float16
F32R = mybir.dt.float32r
AX = mybir.AxisListType
AOP = mybir.AluOpType
AF = mybir.ActivationFunctionType


def _issue_loads(nc, pools, src, tdt, trange=None, eng=None):
    """Issue row-block DMA loads for a [1024, 1024] DRAM tensor. eng=gpsimd
    routes through SWDGE, whose descriptor generation runs on the Pool
    engine in parallel with the serialized shared-HWDGE issue path."""
    stage = pools["stage"]
    eng = eng or nc.sync
    tiles = []
    for t_ in trange if trange is not None else range(NT):
        xf = stage.tile([P, T], tdt, tag="ldf32", bufs=OPTS["stage_bufs"])
        src_ap = src[ts(t_, P), :]
        if tdt != F32:
            src_ap = src_ap.bitcast(tdt)
        if OPTS["split_first"] and t_ == 0 and trange is None:
            eng.dma_start(xf[:, ds(0, 512)], src_ap[:, ds(0, 512)])
            eng.dma_start(xf[:, ds(512, 512)], src_ap[:, ds(512, 512)])
        else:
            eng.dma_start(xf[:], src_ap)
        tiles.append(xf)
    return tiles


def _cold_stage(nc, pools, q_d0, k_d0, w_d, tdt):
    """Batch-0 cold start: q row-blocks, W column-blocks and keys row-blocks
    issued in an order matched to the serialized DMA bandwidth — q0 (split
    for an earlier first transpose), W0/W1 (needed by the fused et0/et1
    accumulation), the rest of q, then W2..7 interleaved with keys so each
    lands just before its consumer."""
    stage = pools["stage"]
    qtiles = []
    xf = stage.tile([P, T], tdt, tag="ldf32", bufs=OPTS["stage_bufs"])
    src = q_d0[ts(0, P), :]
    if tdt != F32:
        src = src.bitcast(tdt)
    ns = OPTS["q0_split"]
    cw = T // ns
    for c in range(ns):
        nc.sync.dma_start(xf[:, ds(c * cw, cw)], src[:, ds(c * cw, cw)])
    qtiles.append(xf)
    W_t = [None] * NT
    ktiles = []
    for kind, i in OPTS["cold_order"]:
        if kind == "W":
            W_t[i] = _issue_w_load(nc, pools, w_d, i)
        elif kind == "q":
            qtiles += _issue_loads(nc, pools, q_d0, tdt, trange=[i])
        else:
            ktiles += _issue_loads(nc, pools, k_d0, tdt, trange=[i])
    return qtiles, W_t, ktiles


def _fused_qT_et01(nc, pools, qtiles, W_t, qdrain):
    """Transpose the q row-blocks and accumulate phase-1 et0/et1 in the same
    loop, one dt behind each transpose (so the mm never waits on the psum
    drain), turning the DMA-bound staging window into PE work. Dummy
    identity transposes (warm) fill the DMA waits so the tensor-engine
    clock ramps to full and the fused mms don't run at the mid p-state."""
    psA = pools["psA"]
    qT = pools["xt"].tile([P, NT, T], F32R, tag="xT")
    ps_e0 = psA.tile([P, T], F32, tag="psA")
    ps_e1 = psA.tile([P, T], F32, tag="psA")
    ps_e = [ps_e0, ps_e1]

    def mm(d):
        for eti in range(2):
            for qc in range(2):
                nc.tensor.matmul(
                    ps_e[eti][:, ds(qc * 512, 512)], W_t[eti][:, d, :],
                    qT[:, d, ds(qc * 512, 512)],
                    start=(d == 0), stop=(d == NT - 1),
                )

    lag = OPTS["mm_lag"]
    for t in range(NT):
        _transpose_from(nc, pools, [qtiles[t]], qdrain, xT=qT, t0=t,
                        warm_first=(OPTS["warm_n0"] if t == 0
                                    else OPTS["warm_n"]))
        if t >= lag:
            mm(t - lag)
    for d in range(NT - lag, NT):
        mm(d)
    return qT, ps_e


def _transpose_from(nc, pools, tiles, drain_eng, xT=None, t0=0, pool="xt",
                    warm_first=0, warm_last=0):
    """xT[d, o, blk] = src^T from pre-loaded row-block stage tiles.

    warm_first/warm_last emit redundant PE transposes (identity into the
    block-0 region before the data lands / a repeat of the last block after)
    purely to keep the tensor engine busy through DMA waits so its p-state
    clock ramp doesn't reset. They are idempotent overwrites of regions the
    real transposes (re)write, so the drained data is unchanged."""
    psB = pools["psB"]
    idt = pools["identT"]
    tdt = idt.dtype
    if xT is None:
        xT = pools[pool].tile([P, NT, T], F32R, tag="xT")
    for i, xf in enumerate(tiles):
        t_ = t0 + i
        pst = psB.tile([P, T], F32, tag="psB")
        out_v = pst[:] if tdt == F32 else pst[:].bitcast(tdt)
        for _ in range(warm_first):
            nc.tensor.transpose(out_v[:, ts(0, P)], idt[:], idt[:])
        for b_ in range(NT):
            nc.tensor.transpose(out_v[:, ts(b_, P)], xf[:, ts(b_, P)], idt[:])
        for _ in range(warm_last):
            nc.tensor.transpose(out_v[:, ts(NT - 1, P)],
                                xf[:, ts(NT - 1, P)], idt[:])
        view = pst[:].rearrange("p (o q) -> p o q", q=P)
        if OPTS["split_drain"]:
            # halves drain concurrently on ACT+DVE so the PSUM buffer frees
            # in ~660ns, keeping up with the 640ns transpose cadence
            H = NT // 2
            nc.scalar.copy(xT[:, ds(0, H), ts(t_, P)], view[:, ds(0, H), :])
            nc.vector.tensor_copy(xT[:, ds(H, H), ts(t_, P)], view[:, ds(H, H), :])
        else:
            drain_eng(xT[:, :, ts(t_, P)], view)
    return xT


def _issue_w_load(nc, pools, w_d, et):
    """One W column-block as lhsT layout [d%128, d//128, e]. Separate tiles
    per 128-col block keep the dependency regions disjoint (a single tile's
    strided column writes get bounding-box-merged, making phase 1 et=0 wait
    on several W DMAs instead of one)."""
    W_t = pools["wres"].tile([P, NT, P], F32R, tag=f"W{et}")
    nc.sync.dma_start(
        W_t[:],
        w_d[:, ts(et, P)].rearrange("(o p) e -> p o e", p=P).bitcast(F32R),
    )
    return W_t


def _issue_w_loads(nc, pools, w_d):
    return [_issue_w_load(nc, pools, w_d, et) for et in range(NT)]


def _phase1(nc, pools, W_t, qT, kt_work=None, pre=None):
    """qWT[e, et, q] = W^T @ query^T, single-pass f32r.

    kt_work: optional {et_step: [thunk, ...]} of PE-side staging work (keys
    row-block transposes) interleaved after the given et steps, so the kT
    psum drains overlap the phase-1 matmul bursts instead of stalling the PE
    in a dedicated transpose block afterwards."""
    qwt_pool, psA = pools["qwt"], pools["psA"]
    qWT = qwt_pool.tile([P, NT, T], F32R, tag="qWT")
    et0 = 0
    drains = []
    if pre is not None:
        # et0/et1 already accumulated during the fused staging loop
        for eti, ps in enumerate(pre):
            drains.append(nc.vector.tensor_copy(qWT[:, eti, :], ps[:]))
        et0 = len(pre)
    for et in range(et0, NT):
        ps = psA.tile([P, T], F32, tag="psA")
        for dt_ in range(NT):
            lw = W_t[et][:, dt_, :]
            for qc in range(2):
                nc.tensor.matmul(
                    ps[:, ds(qc * 512, 512)], lw,
                    qT[:, dt_, ds(qc * 512, 512)],
                    start=(dt_ == 0), stop=(dt_ == NT - 1),
                )
        drains.append(nc.vector.tensor_copy(qWT[:, et, :], ps[:]))
        if kt_work:
            for thunk in kt_work.get(et, ()):
                thunk()
    return qWT, drains


def _p2_step(nc, pools, b, s_d, qWT, kT, mrep, qt_):
    """One qt row-block of phase 2: logits matmuls (chunk-outer so chunk 0
    starts before the last kT drains land), then the softmax chain."""
    soft, psA = pools["soft"], pools["psA"]
    ps2 = psA.tile([P, T], F32, tag="psA")
    for kc in range(2):
        for et in range(NT):
            nc.tensor.matmul(
                ps2[:, ds(kc * 512, 512)], qWT[:, et, ts(qt_, P)],
                kT[:, et, ds(kc * 512, 512)],
                start=(et == 0), stop=(et == NT - 1),
            )
    # drain + mask add fused: wsb = ps2 + mask (fp32, exact)
    wsb = soft.tile([P, T], F32, tag="wsb")
    nc.vector.tensor_tensor(wsb[:], ps2[:], mrep[:], AOP.add)
    negmax = soft.tile([P, 1], F32, tag="negmax")
    nc.vector.tensor_reduce(negmax[:], wsb[:], axis=AX.X, op=AOP.max, negate=True)
    sumexp = soft.tile([P, 1], F32, tag="sumexp")
    if OPTS["exp_inplace"]:
        expt = wsb
    else:
        expt = soft.tile([P, T], F32, tag="expt")
    nc.scalar.activation(
        expt[:], wsb[:], AF.Exp, bias=negmax[:], scale=1.0, accum_out=sumexp[:]
    )
    recip = soft.tile([P, 1], F32, tag="recip")
    nc.vector.reciprocal(recip[:], sumexp[:])
    if OPTS["s16_direct"]:
        # normalize straight into fp16; score goes to DRAM via SWDGE
        # cast-store (f16 -> f32). abs err <= 5e-4 on scores in [0,1].
        # distinct per-slot tags = distinct SBUF tensors, so the region
        # tracker can't bounding-box-merge adjacent slots' writes
        s16 = pools["sc"].tile(
            [P, T], F16, tag=f"s16_{qt_ % OPTS['s16_bufs']}", bufs=1
        )
        nc.vector.tensor_scalar_mul(s16[:], expt[:], recip[:])
        nc.gpsimd.dma_start(s_d[b, ts(qt_, P), :], s16[:])
        return s16
    nc.vector.tensor_scalar_mul(expt[:], expt[:], recip[:])
    nc.sync.dma_start(s_d[b, ts(qt_, P), :], expt[:])
    s16 = pools["sc"].tile([P, T], F16, tag=f"s16_{qt_}")
    if qt_ >= NT - OPTS["s16_dve_last"]:
        nc.vector.tensor_copy(s16[:], expt[:])
    else:
        nc.scalar.copy(s16[:], expt[:])
    return s16


def _p3_transpose_dma(nc, pools, s16, qt_):
    """Score transpose on the DMA crossbar: one InstDmaTransposeAnt per
    [128, 1024] fp16 row-block, writing the [k, kt, q] SBUF layout directly
    (64 xbar tiles x 14 ns ~= 0.9 us on the shared DMA engines; zero PE/ACT).
    Emitted one pipeline step late (sT_off) so the issuing queue never
    head-blocks waiting for the softmax chain."""
    sT16 = pools["st"].tile(
        [P, NT, P], F16, tag=f"sT16_{qt_ % OPTS['st_bufs']}", bufs=1
    )
    eng = nc.scalar if OPTS["sT_eng"] == "act" else nc.sync
    eng.dma_start(sT16[:], s16[:], transpose=True)
    return sT16


def _p3_transpose(nc, pools, s16, dve_drain=False):
    """Transpose one qt row-block of fp16 scores on the PE via an fp16 PSUM
    bitcast view; drain to a [k, kt, q] SBUF tile. Tail transposes drain on
    DVE so the ACT queue is clear when the next batch's staging drains start."""
    psB = pools["psB"]
    ident16 = pools["ident16"]
    ps = psB.tile([P, T], F32, tag="psB")
    view = ps[:, ds(0, 512)].bitcast(F16)
    for kt_ in range(NT):
        nc.tensor.transpose(view[:, ts(kt_, P)], s16[:, ts(kt_, P)], ident16[:])
    sT16 = pools["st"].tile([P, NT, P], F16, tag="sT16", bufs=OPTS["st_bufs"])
    rview = view.rearrange("p (o q) -> p o q", q=P)
    if dve_drain:
        nc.vector.tensor_copy(sT16[:], rview)
    else:
        nc.scalar.copy(sT16[:], rview)
    return sT16, ps


def _p3_mm(nc, pools, b, c_d, sT16, ps_tr, vals, qt_, last=False):
    """ctx[qt block] = score^T^T @ values, fp16. The two 512-col halves
    accumulate into DIFFERENT PSUM TILES (vc1 borrows the free upper half of
    the transpose scratch) so vc1's matmuls carry no false dependency on the
    vc0 drain; ctx DMAs out per half."""
    cx_pool, psA = pools["cx"], pools["psA"]
    ps3 = psA.tile([P, T], F32, tag="psA")
    if ps_tr is None:
        # no PE-transpose scratch to borrow: take a second psA tile so the
        # vc1 matmuls carry no false WAR on the vc0 drain
        ps_tr = psA.tile([P, T], F32, tag="psA")
    halves = (ps3[:, ds(0, 512)], ps_tr[:, ds(512, 512)])
    if not OPTS["cx_half"]:
        cxt_full = cx_pool.tile([P, T], F32, tag="cx")
    for vc in range(2):
        half = halves[vc]
        for kt_ in range(NT):
            nc.tensor.matmul(
                half, sT16[:, kt_, :], vals[:, kt_, ds(vc * 512, 512)],
                start=(kt_ == 0), stop=(kt_ == NT - 1),
            )
        if last and vc == 1 and OPTS["tail_chunks"] > 1:
            # final drain+DMA in fine chunks on alternating engines so the
            # post-last-matmul critical path is one small chunk, not 512 cols
            ncH = OPTS["tail_chunks"]
            w = 512 // ncH
            cxt = cx_pool.tile([P, 512], F32, tag="cx", bufs=OPTS["cx_bufs"])
            for c in range(ncH):
                if c % 2:
                    nc.scalar.copy(cxt[:, ds(c * w, w)], half[:, ds(c * w, w)])
                else:
                    nc.vector.tensor_copy(cxt[:, ds(c * w, w)],
                                          half[:, ds(c * w, w)])
                nc.sync.dma_start(
                    c_d[b, ts(qt_, P), ds(512 + c * w, w)],
                    cxt[:, ds(c * w, w)])
            continue
        if OPTS["cx_half"]:
            cxt = cx_pool.tile([P, 512], F32, tag="cx", bufs=OPTS["cx_bufs"])
            cview = cxt[:]
        else:
            cview = cxt_full[:, ds(vc * 512, 512)]
        if OPTS["cx_act"] > vc or (last and vc == 1 and OPTS["last_cx_act"]):
            nc.scalar.copy(cview, half)
        else:
            nc.vector.tensor_copy(cview, half)
        cx_dma_eng = nc.scalar if OPTS["cx_eng"] == "act" else nc.sync
        cx_dma_eng.dma_start(c_d[b, ts(qt_, P), ds(vc * 512, 512)], cview)


def _phase23(nc, pools, b, s_d, c_d, qWT, kT, mrep, vals, pe_hooks=None):
    """Software-pipelined phases 2+3: step j runs p2(qt=j), the score
    transpose for qt=j-1, and the ctx matmuls for qt=j-2, keeping the PE fed
    while the ACT/DVE drains of earlier tiles complete.

    pe_hooks: optional {step: [thunk, ...]} of extra work (the next batch's
    staging load issues / qT transposes) emitted right after the p2 step."""
    s16s, sT16s = {}, {}
    MO = OPTS["mm_off"]
    TO = OPTS["sT_off"]
    for j in range(NT + MO):
        if j < NT:
            s16s[j] = _p2_step(nc, pools, b, s_d, qWT, kT, mrep, j)
        if pe_hooks:
            for thunk in pe_hooks.get(j, ()):
                thunk()
        if OPTS["sT_dma"] and TO <= j < NT + TO:
            sT16s[j - TO] = (_p3_transpose_dma(nc, pools, s16s[j - TO], j - TO), None)
        if OPTS["mm_first"] and j >= MO:
            sT16, ps_tr = sT16s[j - MO]
            _p3_mm(nc, pools, b, c_d, sT16, ps_tr, vals, j - MO)
        if not OPTS["sT_dma"] and 2 <= j <= NT + 1:
            sT16s[j - 2] = _p3_transpose(nc, pools, s16s[j - 2],
                                         dve_drain=(j >= NT + 1 - OPTS["tail_dve"]))
        if not OPTS["mm_first"] and j >= MO:
            sT16, ps_tr = sT16s[j - MO]
            _p3_mm(nc, pools, b, c_d, sT16, ps_tr, vals, j - MO,
                   last=(b == NB - 1 and j - MO == NT - 1))


def _load_values(nc, pools, b, v_d, after=None, not_before_ms=0.0):
    """SWDGE cast-load: fp32 DRAM -> fp16 SBUF, no compute engine involved.
    not_before_ms steers the static scheduler's placement; `after` (an
    earlier instruction) optionally gates the first load via a real
    dependency edge (the Pool queue is in-order, so one edge holds all
    eight)."""
    vals = pools["val"].tile([P, NT, T], F16, tag="vals")
    tc = pools["tc"]
    with tc.tile_wait_until(not_before_ms):
        for kt_ in range(NT):
            dma = nc.gpsimd.dma_start(vals[:, kt_, :], v_d[b, ts(kt_, P), :])
            if kt_ == 0 and after is not None:
                bass._add_dep_helper(dma.ins, after.ins, sync=True,
                                     reason="defer values past staging crunch")
    return vals


def _stage_mask(nc, pools, b, m_d):
    stage, small = pools["stage"], pools["small"]
    mf = stage.tile([P, T], F32, tag="ldmask", bufs=1)
    nc.gpsimd.dma_start(mf[:1, :], m_d[b : b + 1, :])
    mrep = small.tile([P, T], F32, tag="mrep")
    nc.gpsimd.partition_broadcast(mrep[:], mf[:1, :])
    return mrep


def build_nc(reps=1):
    nc = bacc.Bacc("TRN2", target_bir_lowering=False, debug=False, num_devices=NCORES)
    q_d = nc.dram_tensor("query", [NB, T, T], F32, kind="ExternalInput")
    k_d = nc.dram_tensor("keys", [NB, T, T], F32, kind="ExternalInput")
    v_d = nc.dram_tensor("values", [NB, T, T], F32, kind="ExternalInput")
    w_d = nc.dram_tensor("W", [T, T], F32, kind="ExternalInput")
    m_d = nc.dram_tensor("mask", [NB, T], F32, kind="ExternalInput")
    s_d = nc.dram_tensor("score", [NB, T, T], F32, kind="ExternalOutput")
    c_d = nc.dram_tensor("ctx", [NB, T, T], F32, kind="ExternalOutput")

    with tile.TileContext(nc) as tc:
        with (
            tc.tile_pool(name="stage", bufs=2) as stage,
            tc.tile_pool(name="wres", bufs=1) as wres,
            tc.tile_pool(name="xt", bufs=1) as xt_pool,
            tc.tile_pool(name="kt", bufs=1) as kt_pool,
            tc.tile_pool(name="qwt", bufs=1) as qwt_pool,
            tc.tile_pool(name="val", bufs=1) as val_pool,
            tc.tile_pool(name="sc", bufs=1) as sc_pool,
            tc.tile_pool(name="soft", bufs=OPTS["soft_bufs"]) as soft,
            tc.tile_pool(name="st", bufs=2) as st_pool,
            tc.tile_pool(name="cx", bufs=OPTS["cx_bufs"]) as cx_pool,
            tc.tile_pool(name="small", bufs=1) as small,
            tc.tile_pool(name="ones", bufs=1) as ones_pool,
        ):
            with (
                tc.tile_pool(name="psA", bufs=2, space="PSUM") as psA,
                tc.tile_pool(name="psB", bufs=2, space="PSUM") as psB,
            ):
                pools = {
                    "tc": tc, "wres": wres,
                    "stage": stage, "xt": xt_pool, "kt": kt_pool,
                    "qwt": qwt_pool,
                    "val": val_pool, "soft": soft, "st": st_pool, "sc": sc_pool,
                    "cx": cx_pool, "small": small,
                    "psA": psA, "psB": psB,
                }
                ident16 = ones_pool.tile([P, P], F16, tag="ident16")
                from concourse.masks import make_identity
                make_identity(nc, ident16[:])
                pools["ident16"] = ident16
                identF = ones_pool.tile([P, P], F32, tag="identF")
                make_identity(nc, identF[:])
                if OPTS["ident_t"] == "f32r":
                    identR = ones_pool.tile([P, P], F32R, tag="identR")
                    nc.scalar.copy(identR[:], identF[:])
                    pools["identT"] = identR
                else:
                    pools["identT"] = identF

                tdt = pools["identT"].dtype

                def mk_qdrain():
                    return lambda d, v, _i=iter(range(NT * NT)): (
                        nc.vector.tensor_copy(d, v)
                        if next(_i) >= NT - OPTS["qtail_dve"]
                        else nc.scalar.copy(d, v)
                    )

                kdrain = lambda d, v: nc.scalar.copy(d, v)
                kt_pool_name = "kt" if OPTS["split_xt"] else "xt"
                KSCHED = OPTS["kt_sched"]

                for _rep in range(reps):
                    prepared = {}
                    for b in range(NB):
                        pre_ps = None
                        if b in prepared:
                            qT, ktiles = prepared.pop(b)
                        elif OPTS["cold_fuse"] and _rep == 0 and b == 0:
                            qtiles, W_t, ktiles = _cold_stage(
                                nc, pools, q_d[b], k_d[b], w_d, tdt)
                            qT, pre_ps = _fused_qT_et01(
                                nc, pools, qtiles, W_t, mk_qdrain())
                        else:
                            cs = OPTS["cold_swdge"]
                            if cs == "q0":
                                qtiles = _issue_loads(
                                    nc, pools, q_d[b], tdt, trange=[0],
                                    eng=nc.gpsimd)
                                qtiles += _issue_loads(
                                    nc, pools, q_d[b], tdt,
                                    trange=range(1, NT))
                            else:
                                qtiles = _issue_loads(
                                    nc, pools, q_d[b], tdt,
                                    eng=nc.gpsimd if "q" in cs else None)
                            if _rep == 0 and b == 0:
                                W_t = _issue_w_loads(nc, pools, w_d)
                            if OPTS["warm_n0"] or OPTS["warm_n"]:
                                qT = None
                                qdr = mk_qdrain()
                                for t in range(NT):
                                    qT = _transpose_from(
                                        nc, pools, [qtiles[t]], qdr,
                                        xT=qT, t0=t,
                                        warm_first=(OPTS["warm_n0"] if t == 0
                                                    else OPTS["warm_n"]))
                            else:
                                qT = _transpose_from(nc, pools, qtiles,
                                                     mk_qdrain())
                            ktiles = _issue_loads(
                                nc, pools, k_d[b], tdt,
                                eng=nc.gpsimd if "k" in cs else None)
                        mrep = _stage_mask(nc, pools, b, m_d)
                        vals = None
                        if OPTS["v_pos"] == "early":
                            nb_ms = (OPTS["v_floor_ms"] if b == 0
                                     else (OPTS["v_floor2_ms"]
                                           or OPTS["v_floor_ms"] * (b + 1)))
                            vals = _load_values(
                                nc, pools, b, v_d, not_before_ms=nb_ms,
                            )
                        if OPTS["kT_inter"]:
                            # keys transposes ride inside the phase-1 et
                            # steps; their psum drains overlap the matmuls
                            kT = pools[kt_pool_name].tile(
                                [P, NT, T], F32R, tag="xT"
                            )
                            kt_work = {
                                et: [
                                    (lambda i=i: _transpose_from(
                                        nc, pools, [ktiles[i]], kdrain,
                                        xT=kT, t0=i))
                                    for i in idxs
                                ]
                                for et, idxs in KSCHED.items()
                            }
                            qWT, p1dr = _phase1(nc, pools, W_t, qT, kt_work,
                                                pre=pre_ps)
                        else:
                            qWT, p1dr = _phase1(nc, pools, W_t, qT, pre=pre_ps)
                            kT = _transpose_from(
                                nc, pools, ktiles, kdrain, pool=kt_pool_name,
                            )
                        if vals is None:
                            vdep = (OPTS["v_dep_et"] if b == 0
                                    else OPTS["v_dep_et_warm"])
                            vals = _load_values(
                                nc, pools, b, v_d,
                                after=p1dr[vdep] if vdep is not None else None,
                            )
                        hooks = {}
                        if OPTS["xbatch"] and b + 1 < NB:
                            nb = b + 1
                            nqtiles_box = []
                            nqT = pools["xt"].tile([P, NT, T], F32R, tag="xT")
                            nqdrain = mk_qdrain()

                            def issue_next_q(_nb=nb, _box=nqtiles_box):
                                _box.extend(_issue_loads(nc, pools, q_d[_nb], tdt))

                            hooks.setdefault(0, []).append(issue_next_q)
                            for i in range(NT):
                                hooks.setdefault(i + 1, []).append(
                                    lambda i=i, _box=nqtiles_box: _transpose_from(
                                        nc, pools, [_box[i]], nqdrain,
                                        xT=nqT, t0=i)
                                )
                            nktiles_box = []

                            def issue_next_k(_nb=nb, _box=nktiles_box):
                                _box.extend(_issue_loads(nc, pools, k_d[_nb], tdt))

                            hooks.setdefault(OPTS["xk_step"], []).append(issue_next_k)
                            prepared[nb] = (nqT, nktiles_box)
                        _phase23(nc, pools, b, s_d, c_d, qWT, kT, mrep, vals,
                                 pe_hooks=hooks or None)

    nc.compile()
    return nc


_nc = None


def _get_nc():
    global _nc
    if _nc is None:
        _nc = build_nc()
    return _nc


def make_in_maps(query, keys, values, W, mask):
    query = np.ascontiguousarray(np.asarray(query, dtype=np.float32))
    keys = np.ascontiguousarray(np.asarray(keys, dtype=np.float32))
    values = np.ascontiguousarray(np.asarray(values, dtype=np.float32))
    W = np.ascontiguousarray(np.asarray(W, dtype=np.float32))
    mask = np.ascontiguousarray(np.asarray(mask, dtype=np.float32))
    in_maps = []
    for c in range(NCORES):
        sl = slice(c * NB, (c + 1) * NB)
        in_maps.append(
            {
                "query": query[sl],
                "keys": keys[sl],
                "values": values[sl],
                "W": W,
                "mask": mask[sl],
            }
        )
    return in_maps


def kernel(query, keys, values, W, mask):
    nc = _get_nc()
    in_maps = make_in_maps(query, keys, values, W, mask)
    res = run_bass_kernel_spmd(nc, in_maps, core_ids=list(range(NCORES)))
    score = np.concatenate([res.results[c]["score"] for c in range(NCORES)], axis=0)
    ctx = np.concatenate([res.results[c]["ctx"] for c in range(NCORES)], axis=0)
    return score, ctx



# revision 89
# speedup vs baseline: 1.0001x; 1.0001x over previous
"""Bass/Tile TRN2 kernel for bilinear-score attention (score softmax + context).

reference:
    qW     = query @ W                      [B, Tq, Dk]
    weight = qW @ keys^T + mask[:, None, :] [B, Tq, Tk]
    score  = softmax(weight, axis=-1)
    ctx    = score @ values                 [B, Tq, Dv]
    returns (score, ctx)

Sharding: data-parallel over batch B=16 across 8 NeuronCores (2 batches/core).

Numerics: single-pass float32r matmuls for both big contractions. The PE runs
f32r at full rate (1 cycle/row) for outputs >= 256 wide, at ~tf32 operand
precision (~13 mantissa bits, measured dot-product relmax ~1.6e-4 over K=128).
The resulting logit noise (sigma ~8e-3 absolute on logits of std ~32) is
tolerable because softmax self-normalizes relative logit errors: score error
~= s*(1-s)*(dl_i - dl_top), so near-one-hot rows cancel almost entirely.
Walrus requires f32r matmul operands to be produced as f32r (rounded on
write): DMA loads into f32r tiles and ACT/DVE copies with f32r outputs both
qualify. The mask add stays exact fp32 during the PSUM drain. Phase 3
(score @ values) runs in fp16 with the score transposed on the PE through an
fp16 PSUM bitcast view. No DRAM scratch round trips anywhere.

Schedule per batch: load q row-blocks + W column-blocks -> PE-transpose qT ->
phase 1 with the kT transposes INTERLEAVED into its et steps (kt_sched, so
the kT psum drains overlap the matmul bursts instead of stalling the PE in a
dedicated block) -> software-pipelined phases 2+3 (step j: logits+softmax for
qt=j, score transpose for qt=j-2, ctx matmuls for qt=j-3). The softmax
normalization writes fp16 directly (s16_direct) and the score goes to DRAM
via an SWDGE cast-store, freeing the ACT engine and the SP hwdge queue.
Batch b+1's staging (q/k load issues + its qT PE transposes) rides inside
phase23(b) via pe_hooks (xbatch), and qT/kT live in SEPARATE pools
(split_xt) so the next batch's qT writes don't serialize against this
batch's kT reads -- together these remove the ~6 us inter-batch staging
bubble. values arrive as fp16 via SWDGE cast-loads; their tile_wait_until
floor steers the STATIC scheduler only (the timeline sim has no consumer for
bass_wait_until_ts), which is still enough to keep them out of the cold
staging DMA window.

PE p-state note: the clock ramp (0.65 -> 1.2 -> 2.4 GHz after 3 us of
continuous execution) resets on idle gaps, so the cold-staging transposes
run at the low state inside DMA shadows. warm_n0/warm_n emit idempotent
re-transposes (identity into the block-0 psum region before data lands, a
repeat of the last block after) to hold the clock up through the DMA waits;
worth ~10 ns end-to-end here but harmless, kept for robustness.

Measured (cost-model timeline, device-verified numerics): 207.5 us/core
(baseline of this optimization round: 213.6 us; original 3-pass fp16 hi/lo:
429 us); score/ctx rel err 7.6e-3 vs fp32 reference (gate 2e-2). PE busy
194.3 us = 92.1%; remaining idle: ~4.3 us cold-start DMA latency, ~4.1 us
tail (last drain+DMA+final sems), ~3.5 us DMA-paced q staging.
Note: the two ctx accumulation halves must live in DIFFERENT PSUM tiles --
halves of one tile carry a false WAR dependency (vc1 matmuls wait on the
vc0 drain); vc1 borrows the free upper half of the score-transpose scratch
tile (or a second psA tile when sT_dma=1).

Tried and rejected (cost-model timeline, this round):
- sT_dma: score transpose on the DMA crossbar (InstDmaTransposeAnt, one call
  per [128,1024] fp16 row-block writing the [k,kt,q] layout directly; 64
  xbar tiles x 14ns). Saves 6.8 us of PE but loses 13+ us to serialized
  queue effects: the transpose's sem wait clears ~4 us after its input mul
  (sem batching), it head-blocks whichever hwdge queue carries it, and the
  extra 28.7 us of DMA_ENGINES occupancy starves the cx drain/DMA chain.
  220.3 us vs 207.5.
- cold_fuse / fuse_ets: accumulating phase-1 et0(/et1) inside the q staging
  loop with reordered W/k loads. Theory says p2(0) could start 44-48 us;
  every variant (2-et, 1-et, with/without PE warming) measured p2(0) at
  54-58 us vs the plain path's 53.3. Root cause (verified by counting mms
  in the 17-54us window): the fused et mms wait on their qT-drain
  semaphores, which clear ~4 us after the producing ACT/DVE copy retires
  (compiled-in sem batching), so the "pre-done" accumulation actually
  executes inside phase-1-proper anyway and only the load reorder's DMA
  delay remains. 208.9-214.1 us.
- tail_chunks>1, cx_eng=act, mm_first, split_drain, split_first, deeper
  mm_off: all regress by 0.4-30 us (queue head-blocking or buffer churn).
- cold_swdge: cold q/k staging loads via SWDGE to parallelize descriptor
  generation past the shared-HWDGE 625ns serialization. 218-229 us — the
  slower Pool issue cadence (994+0.34/desc) and dge delays reorder the
  DMA_ENGINES grants and starve the transpose pipeline.

Known-broken paths (device runtime or walrus): DVE tensor_tensor_reduce
(INTERNAL error at run), fp16-identity transposes of 32-bit data and any
16/32-bit matmul operand mix (walrus NCC_IBIR034), gpsimd memset on f32r
tiles (NCC_IXCG864), per-block SBUF->SBUF dma_start_transpose for the score
(descriptor-generation bound: 625 ns/call swamps the 224 ns transfer).
"""

import os
import sys

import numpy as np

os.environ.setdefault("JAX_COMPILATION_CACHE_DIR", "/tmp/jax_comp_cache")

for _p in ("/opt/trn_rl_repo",):
    if _p not in sys.path and os.path.isdir(_p):
        sys.path.insert(0, _p)

import concourse.bass as bass  # noqa: E402
import concourse.tile as tile  # noqa: E402
from concourse import bacc, mybir  # noqa: E402
from concourse.bass import ds, ts  # noqa: E402
from concourse.bass_utils import run_bass_kernel_spmd  # noqa: E402

import json as _json

OPTS = {
    "ident_t": "f32r",   # identity dtype for fp32 PE transposes: f32r|f32|f16
    "v_floor_ms": 0.03,  # not_before floor for values SWDGE loads (batch 0)
    "v_floor2_ms": None, # batch-1 floor; None = v_floor_ms * 2
    "stage_bufs": 6,
    "soft_bufs": 2,
    "st_bufs": 3,
    "cx_bufs": 6,
    "tail_dve": 1,
    "s16_dve_last": 1,
    "ktail_dve": 0,
    "qtail_dve": 0,
    "cx_act": 0,
    "mm_off": 3,
    "split_drain": 0,
    "mm_first": 0,
    "last_cx_act": 0,
    "split_first": 0,
    # new knobs
    "sT_dma": 0,         # score transpose on the DMA xbar instead of the PE
    "s16_direct": 1,     # softmax mul writes fp16; score out via SWDGE cast-store
    "split_xt": 1,       # separate qT / kT pools (kills false cross-tensor serialization)
    "s16_bufs": 2,
    "exp_inplace": 1,    # Exp writes back into wsb (saves the expt tile)
    "cx_half": 1,        # cx drain tiles are per-half [P, 512]
    "sT_off": 1,         # pipeline offset for the score-transpose DMA emission
    "sT_eng": "sp",      # hwdge queue for the score-transpose DMA: sp|act
    "cx_eng": "sp",      # hwdge queue for the ctx output DMAs: sp|act
    "tail_chunks": 1,    # final ctx half drains/DMAs in this many chunks
    "mask_f16": 1,       # fp16 mask broadcast tile (saves 2KB/partition)
    "q_hiprio": 0,       # batch-0 q loads at scheduler priority 0
    "warm_f16": 1,       # warm_first dummies use the fp16 identity
    "fuse_ets": 1,       # how many phase-1 ets accumulate inside q staging
    "kT_inter": 1,       # interleave keys transposes into phase-1 et steps
    "kt_sched": {5: [0, 1], 6: [2, 3], 7: [4, 5, 6, 7]},
    "xbatch": 1,         # stage batch b+1 (loads + qT) inside phase23(b)
    "xk_step": 2,        # phase23 step at which the next batch's k loads issue
    "cold_swdge": "",    # route cold-batch staging loads via SWDGE: ""|q|k|qk
    "warm_n0": 10,       # PE-warming dummy transposes before the first q tile
    "warm_n": 6,         # PE-warming dummies between cold staging transposes
    "v_pos": "early",    # values-load emission: early (pre-phase1) | late
    "v_dep_et": 4,       # late mode, cold batch: values wait for this et drain
    "v_dep_et_warm": 0,  # late mode, warm batches: values wait for this et drain
    "cold_fuse": 0,      # batch-0: fuse et0/et1 accumulation into q staging
    "q0_split": 4,       # split of the first q row-block load
    "mm_lag": 2,         # fused et0/et1 mm runs this many tiles behind qT
    "cold_order": [("q", 1), ("W", 0), ("W", 1),
                   ("q", 2), ("q", 3), ("q", 4), ("q", 5), ("q", 6), ("q", 7),
                   ("W", 2), ("k", 0), ("W", 3), ("k", 1), ("W", 4), ("k", 2),
                   ("W", 5), ("k", 3), ("k", 4), ("W", 6), ("k", 5), ("W", 7),
                   ("k", 6), ("k", 7)],
}
if os.environ.get("K_OPTS"):
    OPTS.update(_json.loads(os.environ["K_OPTS"]))
    if isinstance(OPTS.get("kt_sched"), dict):
        OPTS["kt_sched"] = {int(k): v for k, v in OPTS["kt_sched"].items()}

P = 128
T = 1024
NT = T // P  # 8
NB = 2       # batches per core
NCORES = 8
F32 = mybir.dt.float32
F16 = mybir.dt.float16
F32R = mybir.dt.float32r
AX = mybir.AxisListType
AOP = mybir.AluOpType
AF = mybir.ActivationFunctionType


def _issue_loads(nc, pools, src, tdt, trange=None, eng=None):
    """Issue row-block DMA loads for a [1024, 1024] DRAM tensor. eng=gpsimd
    routes through SWDGE, whose descriptor generation runs on the Pool
    engine in parallel with the serialized shared-HWDGE issue path."""
    stage = pools["stage"]
    eng = eng or nc.sync
    tiles = []
    for t_ in trange if trange is not None else range(NT):
        xf = stage.tile([P, T], tdt, tag="ldf32", bufs=OPTS["stage_bufs"])
        src_ap = src[ts(t_, P), :]
        if tdt != F32:
            src_ap = src_ap.bitcast(tdt)
        if OPTS["split_first"] and t_ == 0 and trange is None:
            eng.dma_start(xf[:, ds(0, 512)], src_ap[:, ds(0, 512)])
            eng.dma_start(xf[:, ds(512, 512)], src_ap[:, ds(512, 512)])
        else:
            eng.dma_start(xf[:], src_ap)
        tiles.append(xf)
    return tiles


def _cold_stage(nc, pools, q_d0, k_d0, w_d, tdt):
    """Batch-0 cold start: q row-blocks, W column-blocks and keys row-blocks
    issued in an order matched to the serialized DMA bandwidth — q0 (split
    for an earlier first transpose), W0/W1 (needed by the fused et0/et1
    accumulation), the rest of q, then W2..7 interleaved with keys so each
    lands just before its consumer."""
    stage = pools["stage"]
    qtiles = []
    xf = stage.tile([P, T], tdt, tag="ldf32", bufs=OPTS["stage_bufs"])
    src = q_d0[ts(0, P), :]
    if tdt != F32:
        src = src.bitcast(tdt)
    ns = OPTS["q0_split"]
    cw = T // ns
    for c in range(ns):
        nc.sync.dma_start(xf[:, ds(c * cw, cw)], src[:, ds(c * cw, cw)])
    qtiles.append(xf)
    W_t = [None] * NT
    ktiles = []
    for kind, i in OPTS["cold_order"]:
        if kind == "W":
            W_t[i] = _issue_w_load(nc, pools, w_d, i)
        elif kind == "q":
            qtiles += _issue_loads(nc, pools, q_d0, tdt, trange=[i])
        else:
            ktiles += _issue_loads(nc, pools, k_d0, tdt, trange=[i])
    return qtiles, W_t, ktiles


def _fused_qT_et01(nc, pools, qtiles, W_t, qdrain):
    """Transpose the q row-blocks and accumulate phase-1 et0/et1 in the same
    loop, one dt behind each transpose (so the mm never waits on the psum
    drain), turning the DMA-bound staging window into PE work. Dummy
    identity transposes (warm) fill the DMA waits so the tensor-engine
    clock ramps to full and the fused mms don't run at the mid p-state."""
    psA = pools["psA"]
    qT = pools["xt"].tile([P, NT, T], F32R, tag="xT")
    n_ets = OPTS["fuse_ets"]
    ps_e0 = psA.tile([P, T], F32, tag="psA")
    ps_e = [ps_e0]
    if n_ets > 1:
        ps_e1 = psA.tile([P, T], F32, tag="psA")
        ps_e.append(ps_e1)

    def mm(d):
        for eti in range(n_ets):
            for qc in range(2):
                nc.tensor.matmul(
                    ps_e[eti][:, ds(qc * 512, 512)], W_t[eti][:, d, :],
                    qT[:, d, ds(qc * 512, 512)],
                    start=(d == 0), stop=(d == NT - 1),
                )

    lag = OPTS["mm_lag"]
    for t in range(NT):
        _transpose_from(nc, pools, [qtiles[t]], qdrain, xT=qT, t0=t,
                        warm_first=(OPTS["warm_n0"] if t == 0
                                    else OPTS["warm_n"]))
        if t >= lag:
            mm(t - lag)
    for d in range(NT - lag, NT):
        mm(d)
    return qT, ps_e


def _transpose_from(nc, pools, tiles, drain_eng, xT=None, t0=0, pool="xt",
                    warm_first=0, warm_last=0):
    """xT[d, o, blk] = src^T from pre-loaded row-block stage tiles.

    warm_first/warm_last emit redundant PE transposes (identity into the
    block-0 region before the data lands / a repeat of the last block after)
    purely to keep the tensor engine busy through DMA waits so its p-state
    clock ramp doesn't reset. They are idempotent overwrites of regions the
    real transposes (re)write, so the drained data is unchanged."""
    psB = pools["psB"]
    idt = pools["identT"]
    tdt = idt.dtype
    if xT is None:
        xT = pools[pool].tile([P, NT, T], F32R, tag="xT")
    for i, xf in enumerate(tiles):
        t_ = t0 + i
        pst = psB.tile([P, T], F32, tag="psB")
        out_v = pst[:] if tdt == F32 else pst[:].bitcast(tdt)
        id16 = pools["ident16"]
        w_out = pst[:, ds(0, 2 * P)].bitcast(F16) if OPTS["warm_f16"] else None
        for _ in range(warm_first):
            if w_out is not None:
                # fp16 identity self-transpose: ready earliest (no identR
                # dependency), 1.0 c/r; overwritten by the real block 0
                nc.tensor.transpose(w_out[:, ds(0, P)], id16[:], id16[:])
            else:
                nc.tensor.transpose(out_v[:, ts(0, P)], idt[:], idt[:])
        for b_ in range(NT):
            nc.tensor.transpose(out_v[:, ts(b_, P)], xf[:, ts(b_, P)], idt[:])
        for _ in range(warm_last):
            nc.tensor.transpose(out_v[:, ts(NT - 1, P)],
                                xf[:, ts(NT - 1, P)], idt[:])
        view = pst[:].rearrange("p (o q) -> p o q", q=P)
        if OPTS["split_drain"]:
            # halves drain concurrently on ACT+DVE so the PSUM buffer frees
            # in ~660ns, keeping up with the 640ns transpose cadence
            H = NT // 2
            nc.scalar.copy(xT[:, ds(0, H), ts(t_, P)], view[:, ds(0, H), :])
            nc.vector.tensor_copy(xT[:, ds(H, H), ts(t_, P)], view[:, ds(H, H), :])
        else:
            drain_eng(xT[:, :, ts(t_, P)], view)
    return xT


def _issue_w_load(nc, pools, w_d, et):
    """One W column-block as lhsT layout [d%128, d//128, e]. Separate tiles
    per 128-col block keep the dependency regions disjoint (a single tile's
    strided column writes get bounding-box-merged, making phase 1 et=0 wait
    on several W DMAs instead of one)."""
    W_t = pools["wres"].tile([P, NT, P], F32R, tag=f"W{et}")
    nc.sync.dma_start(
        W_t[:],
        w_d[:, ts(et, P)].rearrange("(o p) e -> p o e", p=P).bitcast(F32R),
    )
    return W_t


def _issue_w_loads(nc, pools, w_d):
    return [_issue_w_load(nc, pools, w_d, et) for et in range(NT)]


def _phase1(nc, pools, W_t, qT, kt_work=None, pre=None):
    """qWT[e, et, q] = W^T @ query^T, single-pass f32r.

    kt_work: optional {et_step: [thunk, ...]} of PE-side staging work (keys
    row-block transposes) interleaved after the given et steps, so the kT
    psum drains overlap the phase-1 matmul bursts instead of stalling the PE
    in a dedicated transpose block afterwards."""
    qwt_pool, psA = pools["qwt"], pools["psA"]
    qWT = qwt_pool.tile([P, NT, T], F32R, tag="qWT")
    et0 = 0
    drains = []
    if pre is not None:
        # et0/et1 already accumulated during the fused staging loop
        for eti, ps in enumerate(pre):
            drains.append(nc.vector.tensor_copy(qWT[:, eti, :], ps[:]))
        et0 = len(pre)
    for et in range(et0, NT):
        ps = psA.tile([P, T], F32, tag="psA")
        for dt_ in range(NT):
            lw = W_t[et][:, dt_, :]
            for qc in range(2):
                nc.tensor.matmul(
                    ps[:, ds(qc * 512, 512)], lw,
                    qT[:, dt_, ds(qc * 512, 512)],
                    start=(dt_ == 0), stop=(dt_ == NT - 1),
                )
        drains.append(nc.vector.tensor_copy(qWT[:, et, :], ps[:]))
        if kt_work:
            for thunk in kt_work.get(et, ()):
                thunk()
    return qWT, drains


def _p2_step(nc, pools, b, s_d, qWT, kT, mrep, qt_):
    """One qt row-block of phase 2: logits matmuls (chunk-outer so chunk 0
    starts before the last kT drains land), then the softmax chain."""
    soft, psA = pools["soft"], pools["psA"]
    ps2 = psA.tile([P, T], F32, tag="psA")
    for kc in range(2):
        for et in range(NT):
            nc.tensor.matmul(
                ps2[:, ds(kc * 512, 512)], qWT[:, et, ts(qt_, P)],
                kT[:, et, ds(kc * 512, 512)],
                start=(et == 0), stop=(et == NT - 1),
            )
    # drain + mask add fused: wsb = ps2 + mask (fp32, exact)
    wsb = soft.tile([P, T], F32, tag="wsb")
    nc.vector.tensor_tensor(wsb[:], ps2[:], mrep[:], AOP.add)
    negmax = soft.tile([P, 1], F32, tag="negmax")
    nc.vector.tensor_reduce(negmax[:], wsb[:], axis=AX.X, op=AOP.max, negate=True)
    sumexp = soft.tile([P, 1], F32, tag="sumexp")
    if OPTS["exp_inplace"]:
        expt = wsb
    else:
        expt = soft.tile([P, T], F32, tag="expt")
    nc.scalar.activation(
        expt[:], wsb[:], AF.Exp, bias=negmax[:], scale=1.0, accum_out=sumexp[:]
    )
    recip = soft.tile([P, 1], F32, tag="recip")
    nc.vector.reciprocal(recip[:], sumexp[:])
    if OPTS["s16_direct"]:
        # normalize straight into fp16; score goes to DRAM via SWDGE
        # cast-store (f16 -> f32). abs err <= 5e-4 on scores in [0,1].
        # distinct per-slot tags = distinct SBUF tensors, so the region
        # tracker can't bounding-box-merge adjacent slots' writes
        s16 = pools["sc"].tile(
            [P, T], F16, tag=f"s16_{qt_ % OPTS['s16_bufs']}", bufs=1
        )
        nc.vector.tensor_scalar_mul(s16[:], expt[:], recip[:])
        nc.gpsimd.dma_start(s_d[b, ts(qt_, P), :], s16[:])
        return s16
    nc.vector.tensor_scalar_mul(expt[:], expt[:], recip[:])
    nc.sync.dma_start(s_d[b, ts(qt_, P), :], expt[:])
    s16 = pools["sc"].tile([P, T], F16, tag=f"s16_{qt_}")
    if qt_ >= NT - OPTS["s16_dve_last"]:
        nc.vector.tensor_copy(s16[:], expt[:])
    else:
        nc.scalar.copy(s16[:], expt[:])
    return s16


def _p3_transpose_dma(nc, pools, s16, qt_):
    """Score transpose on the DMA crossbar: one InstDmaTransposeAnt per
    [128, 1024] fp16 row-block, writing the [k, kt, q] SBUF layout directly
    (64 xbar tiles x 14 ns ~= 0.9 us on the shared DMA engines; zero PE/ACT).
    Emitted one pipeline step late (sT_off) so the issuing queue never
    head-blocks waiting for the softmax chain."""
    sT16 = pools["st"].tile(
        [P, NT, P], F16, tag=f"sT16_{qt_ % OPTS['st_bufs']}", bufs=1
    )
    eng = nc.scalar if OPTS["sT_eng"] == "act" else nc.sync
    eng.dma_start(sT16[:], s16[:], transpose=True)
    return sT16


def _p3_transpose(nc, pools, s16, dve_drain=False):
    """Transpose one qt row-block of fp16 scores on the PE via an fp16 PSUM
    bitcast view; drain to a [k, kt, q] SBUF tile. Tail transposes drain on
    DVE so the ACT queue is clear when the next batch's staging drains start."""
    psB = pools["psB"]
    ident16 = pools["ident16"]
    ps = psB.tile([P, T], F32, tag="psB")
    view = ps[:, ds(0, 512)].bitcast(F16)
    for kt_ in range(NT):
        nc.tensor.transpose(view[:, ts(kt_, P)], s16[:, ts(kt_, P)], ident16[:])
    sT16 = pools["st"].tile([P, NT, P], F16, tag="sT16", bufs=OPTS["st_bufs"])
    rview = view.rearrange("p (o q) -> p o q", q=P)
    if dve_drain:
        nc.vector.tensor_copy(sT16[:], rview)
    else:
        nc.scalar.copy(sT16[:], rview)
    return sT16, ps


def _p3_mm(nc, pools, b, c_d, sT16, ps_tr, vals, qt_, last=False):
    """ctx[qt block] = score^T^T @ values, fp16. The two 512-col halves
    accumulate into DIFFERENT PSUM TILES (vc1 borrows the free upper half of
    the transpose scratch) so vc1's matmuls carry no false dependency on the
    vc0 drain; ctx DMAs out per half."""
    cx_pool, psA = pools["cx"], pools["psA"]
    ps3 = psA.tile([P, T], F32, tag="psA")
    if ps_tr is None:
        # no PE-transpose scratch to borrow: take a second psA tile so the
        # vc1 matmuls carry no false WAR on the vc0 drain
        ps_tr = psA.tile([P, T], F32, tag="psA")
    halves = (ps3[:, ds(0, 512)], ps_tr[:, ds(512, 512)])
    if not OPTS["cx_half"]:
        cxt_full = cx_pool.tile([P, T], F32, tag="cx")
    for vc in range(2):
        half = halves[vc]
        for kt_ in range(NT):
            nc.tensor.matmul(
                half, sT16[:, kt_, :], vals[:, kt_, ds(vc * 512, 512)],
                start=(kt_ == 0), stop=(kt_ == NT - 1),
            )
        if last and vc == 1 and OPTS["tail_chunks"] > 1:
            # final drain+DMA in fine chunks on alternating engines so the
            # post-last-matmul critical path is one small chunk, not 512 cols
            ncH = OPTS["tail_chunks"]
            w = 512 // ncH
            for c in range(ncH):
                # one tile per chunk: distinct tensors, so the chunk DMAs
                # can't get serialized by region-merged write tracking
                cxt = cx_pool.tile([P, w], F32, tag=f"cxtl{c}", bufs=1)
                if c % 2:
                    nc.scalar.copy(cxt[:], half[:, ds(c * w, w)])
                else:
                    nc.vector.tensor_copy(cxt[:], half[:, ds(c * w, w)])
                nc.sync.dma_start(
                    c_d[b, ts(qt_, P), ds(512 + c * w, w)], cxt[:])
            continue
        if OPTS["cx_half"]:
            cxt = cx_pool.tile([P, 512], F32, tag="cx", bufs=OPTS["cx_bufs"])
            cview = cxt[:]
        else:
            cview = cxt_full[:, ds(vc * 512, 512)]
        if OPTS["cx_act"] > vc or (last and vc == 1 and OPTS["last_cx_act"]):
            nc.scalar.copy(cview, half)
        else:
            nc.vector.tensor_copy(cview, half)
        cx_dma_eng = nc.scalar if OPTS["cx_eng"] == "act" else nc.sync
        cx_dma_eng.dma_start(c_d[b, ts(qt_, P), ds(vc * 512, 512)], cview)


def _phase23(nc, pools, b, s_d, c_d, qWT, kT, mrep, vals, pe_hooks=None):
    """Software-pipelined phases 2+3: step j runs p2(qt=j), the score
    transpose for qt=j-1, and the ctx matmuls for qt=j-2, keeping the PE fed
    while the ACT/DVE drains of earlier tiles complete.

    pe_hooks: optional {step: [thunk, ...]} of extra work (the next batch's
    staging load issues / qT transposes) emitted right after the p2 step."""
    s16s, sT16s = {}, {}
    MO = OPTS["mm_off"]
    TO = OPTS["sT_off"]
    for j in range(NT + MO):
        if j < NT:
            s16s[j] = _p2_step(nc, pools, b, s_d, qWT, kT, mrep, j)
        if pe_hooks:
            for thunk in pe_hooks.get(j, ()):
                thunk()
        if OPTS["sT_dma"] and TO <= j < NT + TO:
            sT16s[j - TO] = (_p3_transpose_dma(nc, pools, s16s[j - TO], j - TO), None)
        if OPTS["mm_first"] and j >= MO:
            sT16, ps_tr = sT16s[j - MO]
            _p3_mm(nc, pools, b, c_d, sT16, ps_tr, vals, j - MO)
        if not OPTS["sT_dma"] and 2 <= j <= NT + 1:
            sT16s[j - 2] = _p3_transpose(nc, pools, s16s[j - 2],
                                         dve_drain=(j >= NT + 1 - OPTS["tail_dve"]))
        if not OPTS["mm_first"] and j >= MO:
            sT16, ps_tr = sT16s[j - MO]
            _p3_mm(nc, pools, b, c_d, sT16, ps_tr, vals, j - MO,
                   last=(b == NB - 1 and j - MO == NT - 1))


def _load_values(nc, pools, b, v_d, after=None, not_before_ms=0.0):
    """SWDGE cast-load: fp32 DRAM -> fp16 SBUF, no compute engine involved.
    not_before_ms steers the static scheduler's placement; `after` (an
    earlier instruction) optionally gates the first load via a real
    dependency edge (the Pool queue is in-order, so one edge holds all
    eight)."""
    vals = pools["val"].tile([P, NT, T], F16, tag="vals")
    tc = pools["tc"]
    with tc.tile_wait_until(not_before_ms):
        for kt_ in range(NT):
            dma = nc.gpsimd.dma_start(vals[:, kt_, :], v_d[b, ts(kt_, P), :])
            if kt_ == 0 and after is not None:
                bass._add_dep_helper(dma.ins, after.ins, sync=True,
                                     reason="defer values past staging crunch")
    return vals


def _stage_mask(nc, pools, b, m_d):
    """Mask broadcast tile in fp16 (cast on the SWDGE load): costs <=5e-4
    absolute on logits of std ~32 — negligible — and halves the tile."""
    stage, small = pools["stage"], pools["small"]
    mdt = F16 if OPTS["mask_f16"] else F32
    mf = stage.tile([P, T], mdt, tag="ldmask", bufs=1)
    nc.gpsimd.dma_start(mf[:1, :], m_d[b : b + 1, :])
    mrep = small.tile([P, T], mdt, tag="mrep")
    nc.gpsimd.partition_broadcast(mrep[:], mf[:1, :])
    return mrep


def build_nc(reps=1):
    nc = bacc.Bacc("TRN2", target_bir_lowering=False, debug=False, num_devices=NCORES)
    q_d = nc.dram_tensor("query", [NB, T, T], F32, kind="ExternalInput")
    k_d = nc.dram_tensor("keys", [NB, T, T], F32, kind="ExternalInput")
    v_d = nc.dram_tensor("values", [NB, T, T], F32, kind="ExternalInput")
    w_d = nc.dram_tensor("W", [T, T], F32, kind="ExternalInput")
    m_d = nc.dram_tensor("mask", [NB, T], F32, kind="ExternalInput")
    s_d = nc.dram_tensor("score", [NB, T, T], F32, kind="ExternalOutput")
    c_d = nc.dram_tensor("ctx", [NB, T, T], F32, kind="ExternalOutput")

    with tile.TileContext(nc) as tc:
        with (
            tc.tile_pool(name="stage", bufs=2) as stage,
            tc.tile_pool(name="wres", bufs=1) as wres,
            tc.tile_pool(name="xt", bufs=1) as xt_pool,
            tc.tile_pool(name="kt", bufs=1) as kt_pool,
            tc.tile_pool(name="qwt", bufs=1) as qwt_pool,
            tc.tile_pool(name="val", bufs=1) as val_pool,
            tc.tile_pool(name="sc", bufs=1) as sc_pool,
            tc.tile_pool(name="soft", bufs=OPTS["soft_bufs"]) as soft,
            tc.tile_pool(name="st", bufs=2) as st_pool,
            tc.tile_pool(name="cx", bufs=OPTS["cx_bufs"]) as cx_pool,
            tc.tile_pool(name="small", bufs=1) as small,
            tc.tile_pool(name="ones", bufs=1) as ones_pool,
        ):
            with (
                tc.tile_pool(name="psA", bufs=2, space="PSUM") as psA,
                tc.tile_pool(name="psB", bufs=2, space="PSUM") as psB,
            ):
                pools = {
                    "tc": tc, "wres": wres,
                    "stage": stage, "xt": xt_pool, "kt": kt_pool,
                    "qwt": qwt_pool,
                    "val": val_pool, "soft": soft, "st": st_pool, "sc": sc_pool,
                    "cx": cx_pool, "small": small,
                    "psA": psA, "psB": psB,
                }
                ident16 = ones_pool.tile([P, P], F16, tag="ident16")
                from concourse.masks import make_identity
                make_identity(nc, ident16[:])
                pools["ident16"] = ident16
                identF = ones_pool.tile([P, P], F32, tag="identF")
                make_identity(nc, identF[:])
                if OPTS["ident_t"] == "f32r":
                    identR = ones_pool.tile([P, P], F32R, tag="identR")
                    # DVE, not ACT: the ACT queue is busy with the 1.3us
                    # activation-table load at kernel start, and identR
                    # gates the first (warming) PE transposes
                    nc.vector.tensor_copy(identR[:], identF[:])
                    pools["identT"] = identR
                else:
                    pools["identT"] = identF

                tdt = pools["identT"].dtype

                def mk_qdrain():
                    return lambda d, v, _i=iter(range(NT * NT)): (
                        nc.vector.tensor_copy(d, v)
                        if next(_i) >= NT - OPTS["qtail_dve"]
                        else nc.scalar.copy(d, v)
                    )

                kdrain = lambda d, v: nc.scalar.copy(d, v)
                kt_pool_name = "kt" if OPTS["split_xt"] else "xt"
                KSCHED = OPTS["kt_sched"]

                for _rep in range(reps):
                    prepared = {}
                    for b in range(NB):
                        pre_ps = None
                        if b in prepared:
                            qT, ktiles = prepared.pop(b)
                        elif OPTS["cold_fuse"] and _rep == 0 and b == 0:
                            qtiles, W_t, ktiles = _cold_stage(
                                nc, pools, q_d[b], k_d[b], w_d, tdt)
                            qT, pre_ps = _fused_qT_et01(
                                nc, pools, qtiles, W_t, mk_qdrain())
                        else:
                            cs = OPTS["cold_swdge"]
                            hp = (tc.high_priority() if OPTS["q_hiprio"]
                                  and b == 0 and _rep == 0 else None)
                            if hp is not None:
                                hp.__enter__()
                            if cs == "q0":
                                qtiles = _issue_loads(
                                    nc, pools, q_d[b], tdt, trange=[0],
                                    eng=nc.gpsimd)
                                qtiles += _issue_loads(
                                    nc, pools, q_d[b], tdt,
                                    trange=range(1, NT))
                            else:
                                qtiles = _issue_loads(
                                    nc, pools, q_d[b], tdt,
                                    eng=nc.gpsimd if "q" in cs else None)
                            if hp is not None:
                                hp.__exit__(None, None, None)
                            if _rep == 0 and b == 0:
                                W_t = _issue_w_loads(nc, pools, w_d)
                            if OPTS["warm_n0"] or OPTS["warm_n"]:
                                qT = None
                                qdr = mk_qdrain()
                                for t in range(NT):
                                    qT = _transpose_from(
                                        nc, pools, [qtiles[t]], qdr,
                                        xT=qT, t0=t,
                                        warm_first=(OPTS["warm_n0"] if t == 0
                                                    else OPTS["warm_n"]))
                            else:
                                qT = _transpose_from(nc, pools, qtiles,
                                                     mk_qdrain())
                            ktiles = _issue_loads(
                                nc, pools, k_d[b], tdt,
                                eng=nc.gpsimd if "k" in cs else None)
                        mrep = _stage_mask(nc, pools, b, m_d)
                        vals = None
                        if OPTS["v_pos"] == "early":
                            nb_ms = (OPTS["v_floor_ms"] if b == 0
                                     else (OPTS["v_floor2_ms"]
                                           or OPTS["v_floor_ms"] * (b + 1)))
                            vals = _load_values(
                                nc, pools, b, v_d, not_before_ms=nb_ms,
                            )
                        if OPTS["kT_inter"]:
                            # keys transposes ride inside the phase-1 et
                            # steps; their psum drains overlap the matmuls
                            kT = pools[kt_pool_name].tile(
                                [P, NT, T], F32R, tag="xT"
                            )
                            kt_work = {
                                et: [
                                    (lambda i=i: _transpose_from(
                                        nc, pools, [ktiles[i]], kdrain,
                                        xT=kT, t0=i))
                                    for i in idxs
                                ]
                                for et, idxs in KSCHED.items()
                            }
                            qWT, p1dr = _phase1(nc, pools, W_t, qT, kt_work,
                                                pre=pre_ps)
                        else:
                            qWT, p1dr = _phase1(nc, pools, W_t, qT, pre=pre_ps)
                            kT = _transpose_from(
                                nc, pools, ktiles, kdrain, pool=kt_pool_name,
                            )
                        if vals is None:
                            vdep = (OPTS["v_dep_et"] if b == 0
                                    else OPTS["v_dep_et_warm"])
                            vals = _load_values(
                                nc, pools, b, v_d,
                                after=p1dr[vdep] if vdep is not None else None,
                            )
                        hooks = {}
                        if OPTS["xbatch"] and b + 1 < NB:
                            nb = b + 1
                            nqtiles_box = []
                            nqT = pools["xt"].tile([P, NT, T], F32R, tag="xT")
                            nqdrain = mk_qdrain()

                            def issue_next_q(_nb=nb, _box=nqtiles_box):
                                _box.extend(_issue_loads(nc, pools, q_d[_nb], tdt))

                            hooks.setdefault(0, []).append(issue_next_q)
                            for i in range(NT):
                                hooks.setdefault(i + 1, []).append(
                                    lambda i=i, _box=nqtiles_box: _transpose_from(
                                        nc, pools, [_box[i]], nqdrain,
                                        xT=nqT, t0=i)
                                )
                            nktiles_box = []

                            def issue_next_k(_nb=nb, _box=nktiles_box):
                                _box.extend(_issue_loads(nc, pools, k_d[_nb], tdt))

                            hooks.setdefault(OPTS["xk_step"], []).append(issue_next_k)
                            prepared[nb] = (nqT, nktiles_box)
                        _phase23(nc, pools, b, s_d, c_d, qWT, kT, mrep, vals,
                                 pe_hooks=hooks or None)

    nc.compile()
    return nc


_nc = None


def _get_nc():
    global _nc
    if _nc is None:
        _nc = build_nc()
    return _nc


def make_in_maps(query, keys, values, W, mask):
    query = np.ascontiguousarray(np.asarray(query, dtype=np.float32))
    keys = np.ascontiguousarray(np.asarray(keys, dtype=np.float32))
    values = np.ascontiguousarray(np.asarray(values, dtype=np.float32))
    W = np.ascontiguousarray(np.asarray(W, dtype=np.float32))
    mask = np.ascontiguousarray(np.asarray(mask, dtype=np.float32))
    in_maps = []
    for c in range(NCORES):
        sl = slice(c * NB, (c + 1) * NB)
        in_maps.append(
            {
                "query": query[sl],
                "keys": keys[sl],
                "values": values[sl],
                "W": W,
                "mask": mask[sl],
            }
        )
    return in_maps


def kernel(query, keys, values, W, mask):
    nc = _get_nc()
    in_maps = make_in_maps(query, keys, values, W, mask)
    res = run_bass_kernel_spmd(nc, in_maps, core_ids=list(range(NCORES)))
    score = np.concatenate([res.results[c]["score"] for c in range(NCORES)], axis=0)
    ctx = np.concatenate([res.results[c]["ctx"] for c in range(NCORES)], axis=0)
    return score, ctx



# revision 94
# speedup vs baseline: 1.0007x; 1.0006x over previous
"""Bass/Tile TRN2 kernel for bilinear-score attention (score softmax + context).

reference:
    qW     = query @ W                      [B, Tq, Dk]
    weight = qW @ keys^T + mask[:, None, :] [B, Tq, Tk]
    score  = softmax(weight, axis=-1)
    ctx    = score @ values                 [B, Tq, Dv]
    returns (score, ctx)

Sharding: data-parallel over batch B=16 across 8 NeuronCores (2 batches/core).

Numerics: single-pass float32r matmuls for both big contractions. The PE runs
f32r at full rate (1 cycle/row) for outputs >= 256 wide, at ~tf32 operand
precision (~13 mantissa bits, measured dot-product relmax ~1.6e-4 over K=128).
The resulting logit noise (sigma ~8e-3 absolute on logits of std ~32) is
tolerable because softmax self-normalizes relative logit errors: score error
~= s*(1-s)*(dl_i - dl_top), so near-one-hot rows cancel almost entirely.
Walrus requires f32r matmul operands to be produced as f32r (rounded on
write): DMA loads into f32r tiles and ACT/DVE copies with f32r outputs both
qualify. The mask add stays exact fp32 during the PSUM drain. Phase 3
(score @ values) runs in fp16 with the score transposed on the PE through an
fp16 PSUM bitcast view. No DRAM scratch round trips anywhere.

Schedule per batch: load q row-blocks + W column-blocks -> PE-transpose qT ->
phase 1 with the kT transposes INTERLEAVED into its et steps (kt_sched, so
the kT psum drains overlap the matmul bursts instead of stalling the PE in a
dedicated block) -> software-pipelined phases 2+3 (step j: logits+softmax for
qt=j, score transpose for qt=j-2, ctx matmuls for qt=j-3). The softmax
normalization writes fp16 directly (s16_direct) and the score goes to DRAM
via an SWDGE cast-store, freeing the ACT engine and the SP hwdge queue.
Batch b+1's staging (q/k load issues + its qT PE transposes) rides inside
phase23(b) via pe_hooks (xbatch), and qT/kT live in SEPARATE pools
(split_xt) so the next batch's qT writes don't serialize against this
batch's kT reads -- together these remove the ~6 us inter-batch staging
bubble. values arrive as fp16 via SWDGE cast-loads; their tile_wait_until
floor steers the STATIC scheduler only (the timeline sim has no consumer for
bass_wait_until_ts), which is still enough to keep them out of the cold
staging DMA window.

PE p-state note: the clock ramp (0.65 -> 1.2 -> 2.4 GHz after 3 us of
continuous execution) resets on idle gaps, so the cold-staging transposes
run at the low state inside DMA shadows. warm_n0/warm_n emit idempotent
re-transposes (identity into the block-0 psum region before data lands, a
repeat of the last block after) to hold the clock up through the DMA waits;
worth ~10 ns end-to-end here but harmless, kept for robustness.

Measured (cost-model timeline, device-verified numerics): 207.47 us/core
(baseline of this optimization round: 213.6 us; original 3-pass fp16 hi/lo:
429 us); score/ctx rel err 7.8e-3 vs fp32 reference (gate 2e-2). PE busy
194.3 us = 92.1%; remaining idle: ~2.5 us cold-start DMA latency, ~4.1 us
tail (last drain+DMA+final sems), ~3 us DMA-paced q staging. The critical
path is [cold DMA prefix ~16.6 us] -> [PE-bound 186.6 us] -> [tail 4.1 us];
the PE-bound span is the algorithmic floor at the required precision (f32r/
fp16 are both 1 cycle/row on this PE; only fp8 DoubleRow is faster and its
3-mantissa-bit operands blow the 2e-2 gate).
Note: the two ctx accumulation halves must live in DIFFERENT PSUM tiles --
halves of one tile carry a false WAR dependency (vc1 matmuls wait on the
vc0 drain); vc1 borrows the free upper half of the score-transpose scratch
tile (or a second psA tile when sT_dma=1).

Tried and rejected (cost-model timeline, this round):
- sT_dma: score transpose on the DMA crossbar (InstDmaTransposeAnt, one call
  per [128,1024] fp16 row-block writing the [k,kt,q] layout directly; 64
  xbar tiles x 14ns). Saves 6.8 us of PE but loses 13+ us to serialized
  queue effects: the transpose's sem wait clears ~4 us after its input mul
  (sem batching), it head-blocks whichever hwdge queue carries it, and the
  extra 28.7 us of DMA_ENGINES occupancy starves the cx drain/DMA chain.
  220.3 us vs 207.5.
- cold_fuse / fuse_ets: accumulating phase-1 et0(/et1) inside the q staging
  loop with reordered W/k loads. Theory says p2(0) could start 44-48 us;
  every variant (2-et, 1-et, with/without PE warming) measured p2(0) at
  54-58 us vs the plain path's 53.3. Root cause (verified by counting mms
  in the 17-54us window): the fused et mms wait on their qT-drain
  semaphores, which clear ~4 us after the producing ACT/DVE copy retires
  (compiled-in sem batching), so the "pre-done" accumulation actually
  executes inside phase-1-proper anyway and only the load reorder's DMA
  delay remains. 208.9-214.1 us.
- tail_chunks>1, cx_eng=act, mm_first, split_drain, split_first, deeper
  mm_off: all regress by 0.4-30 us (queue head-blocking or buffer churn).
- cold_swdge: cold q/k staging loads via SWDGE to parallelize descriptor
  generation past the shared-HWDGE 625ns serialization. 218-229 us — the
  slower Pool issue cadence (994+0.34/desc) and dge delays reorder the
  DMA_ENGINES grants and starve the transpose pipeline.

Known-broken paths (device runtime or walrus): DVE tensor_tensor_reduce
(INTERNAL error at run), fp16-identity transposes of 32-bit data and any
16/32-bit matmul operand mix (walrus NCC_IBIR034), gpsimd memset on f32r
tiles (NCC_IXCG864), per-block SBUF->SBUF dma_start_transpose for the score
(descriptor-generation bound: 625 ns/call swamps the 224 ns transfer).
"""

import os
import sys

import numpy as np

os.environ.setdefault("JAX_COMPILATION_CACHE_DIR", "/tmp/jax_comp_cache")

for _p in ("/opt/trn_rl_repo",):
    if _p not in sys.path and os.path.isdir(_p):
        sys.path.insert(0, _p)

import concourse.bass as bass  # noqa: E402
import concourse.tile as tile  # noqa: E402
from concourse import bacc, mybir  # noqa: E402
from concourse.bass import ds, ts  # noqa: E402
from concourse.bass_utils import run_bass_kernel_spmd  # noqa: E402

import json as _json

OPTS = {
    "ident_t": "f32r",   # identity dtype for fp32 PE transposes: f32r|f32|f16
    "v_floor_ms": 0.03,  # not_before floor for values SWDGE loads (batch 0)
    "v_floor2_ms": None, # batch-1 floor; None = v_floor_ms * 2
    "stage_bufs": 6,
    "soft_bufs": 2,
    "st_bufs": 3,
    "cx_bufs": 6,
    "tail_dve": 1,
    "s16_dve_last": 1,
    "ktail_dve": 0,
    "qtail_dve": 0,
    "cx_act": 0,
    "mm_off": 3,
    "split_drain": 0,
    "mm_first": 0,
    "last_cx_act": 0,
    "split_first": 0,
    # new knobs
    "sT_dma": 0,         # score transpose on the DMA xbar instead of the PE
    "s16_direct": 1,     # softmax mul writes fp16; score out via SWDGE cast-store
    "split_xt": 1,       # separate qT / kT pools (kills false cross-tensor serialization)
    "s16_bufs": 2,
    "exp_inplace": 1,    # Exp writes back into wsb (saves the expt tile)
    "cx_half": 1,        # cx drain tiles are per-half [P, 512]
    "sT_off": 1,         # pipeline offset for the score-transpose DMA emission
    "sT_eng": "sp",      # hwdge queue for the score-transpose DMA: sp|act
    "cx_eng": "sp",      # hwdge queue for the ctx output DMAs: sp|act
    "tail_chunks": 1,    # final ctx half drains/DMAs in this many chunks
    "mask_f16": 1,       # fp16 mask broadcast tile (saves 2KB/partition)
    "q_hiprio": 0,       # batch-0 q loads at scheduler priority 0
    "warm_f16": 1,       # warm_first dummies use the fp16 identity
    "fuse_ets": 1,       # how many phase-1 ets accumulate inside q staging
    "tail_split": 1,     # last qt: vc1 as two independent 256-wide psums
    "kT_inter": 1,       # interleave keys transposes into phase-1 et steps
    "kt_sched": {5: [0, 1], 6: [2, 3], 7: [4, 5, 6, 7]},
    "xbatch": 1,         # stage batch b+1 (loads + qT) inside phase23(b)
    "xk_step": 2,        # phase23 step at which the next batch's k loads issue
    "cold_swdge": "",    # route cold-batch staging loads via SWDGE: ""|q|k|qk
    "warm_n0": 10,       # PE-warming dummy transposes before the first q tile
    "warm_n": 6,         # PE-warming dummies between cold staging transposes
    "v_pos": "early",    # values-load emission: early (pre-phase1) | late
    "v_dep_et": 4,       # late mode, cold batch: values wait for this et drain
    "v_dep_et_warm": 0,  # late mode, warm batches: values wait for this et drain
    "cold_fuse": 0,      # batch-0: fuse et0/et1 accumulation into q staging
    "q0_split": 4,       # split of the first q row-block load
    "mm_lag": 2,         # fused et0/et1 mm runs this many tiles behind qT
    "cold_order": [("q", 1), ("W", 0), ("W", 1),
                   ("q", 2), ("q", 3), ("q", 4), ("q", 5), ("q", 6), ("q", 7),
                   ("W", 2), ("k", 0), ("W", 3), ("k", 1), ("W", 4), ("k", 2),
                   ("W", 5), ("k", 3), ("k", 4), ("W", 6), ("k", 5), ("W", 7),
                   ("k", 6), ("k", 7)],
}
if os.environ.get("K_OPTS"):
    OPTS.update(_json.loads(os.environ["K_OPTS"]))
    if isinstance(OPTS.get("kt_sched"), dict):
        OPTS["kt_sched"] = {int(k): v for k, v in OPTS["kt_sched"].items()}

P = 128
T = 1024
NT = T // P  # 8
NB = 2       # batches per core
NCORES = 8
F32 = mybir.dt.float32
F16 = mybir.dt.float16
F32R = mybir.dt.float32r
AX = mybir.AxisListType
AOP = mybir.AluOpType
AF = mybir.ActivationFunctionType


def _issue_loads(nc, pools, src, tdt, trange=None, eng=None):
    """Issue row-block DMA loads for a [1024, 1024] DRAM tensor. eng=gpsimd
    routes through SWDGE, whose descriptor generation runs on the Pool
    engine in parallel with the serialized shared-HWDGE issue path."""
    stage = pools["stage"]
    eng = eng or nc.sync
    tiles = []
    for t_ in trange if trange is not None else range(NT):
        xf = stage.tile([P, T], tdt, tag="ldf32", bufs=OPTS["stage_bufs"])
        src_ap = src[ts(t_, P), :]
        if tdt != F32:
            src_ap = src_ap.bitcast(tdt)
        if OPTS["split_first"] and t_ == 0 and trange is None:
            eng.dma_start(xf[:, ds(0, 512)], src_ap[:, ds(0, 512)])
            eng.dma_start(xf[:, ds(512, 512)], src_ap[:, ds(512, 512)])
        else:
            eng.dma_start(xf[:], src_ap)
        tiles.append(xf)
    return tiles


def _cold_stage(nc, pools, q_d0, k_d0, w_d, tdt):
    """Batch-0 cold start: q row-blocks, W column-blocks and keys row-blocks
    issued in an order matched to the serialized DMA bandwidth — q0 (split
    for an earlier first transpose), W0/W1 (needed by the fused et0/et1
    accumulation), the rest of q, then W2..7 interleaved with keys so each
    lands just before its consumer."""
    stage = pools["stage"]
    qtiles = []
    xf = stage.tile([P, T], tdt, tag="ldf32", bufs=OPTS["stage_bufs"])
    src = q_d0[ts(0, P), :]
    if tdt != F32:
        src = src.bitcast(tdt)
    ns = OPTS["q0_split"]
    cw = T // ns
    for c in range(ns):
        nc.sync.dma_start(xf[:, ds(c * cw, cw)], src[:, ds(c * cw, cw)])
    qtiles.append(xf)
    W_t = [None] * NT
    ktiles = []
    for kind, i in OPTS["cold_order"]:
        if kind == "W":
            W_t[i] = _issue_w_load(nc, pools, w_d, i)
        elif kind == "q":
            qtiles += _issue_loads(nc, pools, q_d0, tdt, trange=[i])
        else:
            ktiles += _issue_loads(nc, pools, k_d0, tdt, trange=[i])
    return qtiles, W_t, ktiles


def _fused_qT_et01(nc, pools, qtiles, W_t, qdrain):
    """Transpose the q row-blocks and accumulate phase-1 et0/et1 in the same
    loop, one dt behind each transpose (so the mm never waits on the psum
    drain), turning the DMA-bound staging window into PE work. Dummy
    identity transposes (warm) fill the DMA waits so the tensor-engine
    clock ramps to full and the fused mms don't run at the mid p-state."""
    psA = pools["psA"]
    qT = pools["xt"].tile([P, NT, T], F32R, tag="xT")
    n_ets = OPTS["fuse_ets"]
    ps_e0 = psA.tile([P, T], F32, tag="psA")
    ps_e = [ps_e0]
    if n_ets > 1:
        ps_e1 = psA.tile([P, T], F32, tag="psA")
        ps_e.append(ps_e1)

    def mm(d):
        for eti in range(n_ets):
            for qc in range(2):
                nc.tensor.matmul(
                    ps_e[eti][:, ds(qc * 512, 512)], W_t[eti][:, d, :],
                    qT[:, d, ds(qc * 512, 512)],
                    start=(d == 0), stop=(d == NT - 1),
                )

    lag = OPTS["mm_lag"]
    for t in range(NT):
        _transpose_from(nc, pools, [qtiles[t]], qdrain, xT=qT, t0=t,
                        warm_first=(OPTS["warm_n0"] if t == 0
                                    else OPTS["warm_n"]))
        if t >= lag:
            mm(t - lag)
    for d in range(NT - lag, NT):
        mm(d)
    return qT, ps_e


def _transpose_from(nc, pools, tiles, drain_eng, xT=None, t0=0, pool="xt",
                    warm_first=0, warm_last=0):
    """xT[d, o, blk] = src^T from pre-loaded row-block stage tiles.

    warm_first/warm_last emit redundant PE transposes (identity into the
    block-0 region before the data lands / a repeat of the last block after)
    purely to keep the tensor engine busy through DMA waits so its p-state
    clock ramp doesn't reset. They are idempotent overwrites of regions the
    real transposes (re)write, so the drained data is unchanged."""
    psB = pools["psB"]
    idt = pools["identT"]
    tdt = idt.dtype
    if xT is None:
        xT = pools[pool].tile([P, NT, T], F32R, tag="xT")
    for i, xf in enumerate(tiles):
        t_ = t0 + i
        pst = psB.tile([P, T], F32, tag="psB")
        out_v = pst[:] if tdt == F32 else pst[:].bitcast(tdt)
        id16 = pools["ident16"]
        w_out = pst[:, ds(0, 2 * P)].bitcast(F16) if OPTS["warm_f16"] else None
        for _ in range(warm_first):
            if w_out is not None:
                # fp16 identity self-transpose: ready earliest (no identR
                # dependency), 1.0 c/r; overwritten by the real block 0
                nc.tensor.transpose(w_out[:, ds(0, P)], id16[:], id16[:])
            else:
                nc.tensor.transpose(out_v[:, ts(0, P)], idt[:], idt[:])
        for b_ in range(NT):
            nc.tensor.transpose(out_v[:, ts(b_, P)], xf[:, ts(b_, P)], idt[:])
        for _ in range(warm_last):
            nc.tensor.transpose(out_v[:, ts(NT - 1, P)],
                                xf[:, ts(NT - 1, P)], idt[:])
        view = pst[:].rearrange("p (o q) -> p o q", q=P)
        if OPTS["split_drain"]:
            # halves drain concurrently on ACT+DVE so the PSUM buffer frees
            # in ~660ns, keeping up with the 640ns transpose cadence
            H = NT // 2
            nc.scalar.copy(xT[:, ds(0, H), ts(t_, P)], view[:, ds(0, H), :])
            nc.vector.tensor_copy(xT[:, ds(H, H), ts(t_, P)], view[:, ds(H, H), :])
        else:
            drain_eng(xT[:, :, ts(t_, P)], view)
    return xT


def _issue_w_load(nc, pools, w_d, et):
    """One W column-block as lhsT layout [d%128, d//128, e]. Separate tiles
    per 128-col block keep the dependency regions disjoint (a single tile's
    strided column writes get bounding-box-merged, making phase 1 et=0 wait
    on several W DMAs instead of one)."""
    W_t = pools["wres"].tile([P, NT, P], F32R, tag=f"W{et}")
    nc.sync.dma_start(
        W_t[:],
        w_d[:, ts(et, P)].rearrange("(o p) e -> p o e", p=P).bitcast(F32R),
    )
    return W_t


def _issue_w_loads(nc, pools, w_d):
    return [_issue_w_load(nc, pools, w_d, et) for et in range(NT)]


def _phase1(nc, pools, W_t, qT, kt_work=None, pre=None):
    """qWT[e, et, q] = W^T @ query^T, single-pass f32r.

    kt_work: optional {et_step: [thunk, ...]} of PE-side staging work (keys
    row-block transposes) interleaved after the given et steps, so the kT
    psum drains overlap the phase-1 matmul bursts instead of stalling the PE
    in a dedicated transpose block afterwards."""
    qwt_pool, psA = pools["qwt"], pools["psA"]
    qWT = qwt_pool.tile([P, NT, T], F32R, tag="qWT")
    et0 = 0
    drains = []
    if pre is not None:
        # et0/et1 already accumulated during the fused staging loop
        for eti, ps in enumerate(pre):
            drains.append(nc.vector.tensor_copy(qWT[:, eti, :], ps[:]))
        et0 = len(pre)
    for et in range(et0, NT):
        ps = psA.tile([P, T], F32, tag="psA")
        for dt_ in range(NT):
            lw = W_t[et][:, dt_, :]
            for qc in range(2):
                nc.tensor.matmul(
                    ps[:, ds(qc * 512, 512)], lw,
                    qT[:, dt_, ds(qc * 512, 512)],
                    start=(dt_ == 0), stop=(dt_ == NT - 1),
                )
        drains.append(nc.vector.tensor_copy(qWT[:, et, :], ps[:]))
        if kt_work:
            for thunk in kt_work.get(et, ()):
                thunk()
    return qWT, drains


def _p2_step(nc, pools, b, s_d, qWT, kT, mrep, qt_):
    """One qt row-block of phase 2: logits matmuls (chunk-outer so chunk 0
    starts before the last kT drains land), then the softmax chain."""
    soft, psA = pools["soft"], pools["psA"]
    ps2 = psA.tile([P, T], F32, tag="psA")
    for kc in range(2):
        for et in range(NT):
            nc.tensor.matmul(
                ps2[:, ds(kc * 512, 512)], qWT[:, et, ts(qt_, P)],
                kT[:, et, ds(kc * 512, 512)],
                start=(et == 0), stop=(et == NT - 1),
            )
    # drain + mask add fused: wsb = ps2 + mask (fp32, exact)
    wsb = soft.tile([P, T], F32, tag="wsb")
    nc.vector.tensor_tensor(wsb[:], ps2[:], mrep[:], AOP.add)
    negmax = soft.tile([P, 1], F32, tag="negmax")
    nc.vector.tensor_reduce(negmax[:], wsb[:], axis=AX.X, op=AOP.max, negate=True)
    sumexp = soft.tile([P, 1], F32, tag="sumexp")
    if OPTS["exp_inplace"]:
        expt = wsb
    else:
        expt = soft.tile([P, T], F32, tag="expt")
    nc.scalar.activation(
        expt[:], wsb[:], AF.Exp, bias=negmax[:], scale=1.0, accum_out=sumexp[:]
    )
    recip = soft.tile([P, 1], F32, tag="recip")
    nc.vector.reciprocal(recip[:], sumexp[:])
    if OPTS["s16_direct"]:
        # normalize straight into fp16; score goes to DRAM via SWDGE
        # cast-store (f16 -> f32). abs err <= 5e-4 on scores in [0,1].
        # distinct per-slot tags = distinct SBUF tensors, so the region
        # tracker can't bounding-box-merge adjacent slots' writes
        s16 = pools["sc"].tile(
            [P, T], F16, tag=f"s16_{qt_ % OPTS['s16_bufs']}", bufs=1
        )
        nc.vector.tensor_scalar_mul(s16[:], expt[:], recip[:])
        nc.gpsimd.dma_start(s_d[b, ts(qt_, P), :], s16[:])
        return s16
    nc.vector.tensor_scalar_mul(expt[:], expt[:], recip[:])
    nc.sync.dma_start(s_d[b, ts(qt_, P), :], expt[:])
    s16 = pools["sc"].tile([P, T], F16, tag=f"s16_{qt_}")
    if qt_ >= NT - OPTS["s16_dve_last"]:
        nc.vector.tensor_copy(s16[:], expt[:])
    else:
        nc.scalar.copy(s16[:], expt[:])
    return s16


def _p3_transpose_dma(nc, pools, s16, qt_):
    """Score transpose on the DMA crossbar: one InstDmaTransposeAnt per
    [128, 1024] fp16 row-block, writing the [k, kt, q] SBUF layout directly
    (64 xbar tiles x 14 ns ~= 0.9 us on the shared DMA engines; zero PE/ACT).
    Emitted one pipeline step late (sT_off) so the issuing queue never
    head-blocks waiting for the softmax chain."""
    sT16 = pools["st"].tile(
        [P, NT, P], F16, tag=f"sT16_{qt_ % OPTS['st_bufs']}", bufs=1
    )
    eng = nc.scalar if OPTS["sT_eng"] == "act" else nc.sync
    eng.dma_start(sT16[:], s16[:], transpose=True)
    return sT16


def _p3_transpose(nc, pools, s16, dve_drain=False):
    """Transpose one qt row-block of fp16 scores on the PE via an fp16 PSUM
    bitcast view; drain to a [k, kt, q] SBUF tile. Tail transposes drain on
    DVE so the ACT queue is clear when the next batch's staging drains start."""
    psB = pools["psB"]
    ident16 = pools["ident16"]
    ps = psB.tile([P, T], F32, tag="psB")
    view = ps[:, ds(0, 512)].bitcast(F16)
    for kt_ in range(NT):
        nc.tensor.transpose(view[:, ts(kt_, P)], s16[:, ts(kt_, P)], ident16[:])
    sT16 = pools["st"].tile([P, NT, P], F16, tag="sT16", bufs=OPTS["st_bufs"])
    rview = view.rearrange("p (o q) -> p o q", q=P)
    if dve_drain:
        nc.vector.tensor_copy(sT16[:], rview)
    else:
        nc.scalar.copy(sT16[:], rview)
    return sT16, ps


def _p3_mm(nc, pools, b, c_d, sT16, ps_tr, vals, qt_, last=False):
    """ctx[qt block] = score^T^T @ values, fp16. The two 512-col halves
    accumulate into DIFFERENT PSUM TILES (vc1 borrows the free upper half of
    the transpose scratch) so vc1's matmuls carry no false dependency on the
    vc0 drain; ctx DMAs out per half."""
    cx_pool, psA = pools["cx"], pools["psA"]
    ps3 = psA.tile([P, T], F32, tag="psA")
    if ps_tr is None:
        # no PE-transpose scratch to borrow: take a second psA tile so the
        # vc1 matmuls carry no false WAR on the vc0 drain
        ps_tr = psA.tile([P, T], F32, tag="psA")
    halves = (ps3[:, ds(0, 512)], ps_tr[:, ds(512, 512)])
    if not OPTS["cx_half"]:
        cxt_full = cx_pool.tile([P, T], F32, tag="cx")
    vc_range = (0,) if (last and OPTS["tail_split"]) else (0, 1)
    for vc in vc_range:
        half = halves[vc]
        for kt_ in range(NT):
            nc.tensor.matmul(
                half, sT16[:, kt_, :], vals[:, kt_, ds(vc * 512, 512)],
                start=(kt_ == 0), stop=(kt_ == NT - 1),
            )
        if last and vc == 1 and OPTS["tail_chunks"] > 1:
            # final drain+DMA in fine chunks on alternating engines so the
            # post-last-matmul critical path is one small chunk, not 512 cols
            ncH = OPTS["tail_chunks"]
            w = 512 // ncH
            for c in range(ncH):
                # one tile per chunk: distinct tensors, so the chunk DMAs
                # can't get serialized by region-merged write tracking
                cxt = cx_pool.tile([P, w], F32, tag=f"cxtl{c}", bufs=1)
                if c % 2:
                    nc.scalar.copy(cxt[:], half[:, ds(c * w, w)])
                else:
                    nc.vector.tensor_copy(cxt[:], half[:, ds(c * w, w)])
                nc.sync.dma_start(
                    c_d[b, ts(qt_, P), ds(512 + c * w, w)], cxt[:])
            continue
        if OPTS["cx_half"]:
            cxt = cx_pool.tile([P, 512], F32, tag="cx", bufs=OPTS["cx_bufs"])
            cview = cxt[:]
        else:
            cview = cxt_full[:, ds(vc * 512, 512)]
        if OPTS["cx_act"] > vc or (last and vc == 1 and OPTS["last_cx_act"]):
            nc.scalar.copy(cview, half)
        else:
            nc.vector.tensor_copy(cview, half)
        cx_dma_eng = nc.scalar if OPTS["cx_eng"] == "act" else nc.sync
        cx_dma_eng.dma_start(c_d[b, ts(qt_, P), ds(vc * 512, 512)], cview)
    if last and OPTS["tail_split"]:
        # final half as two independent 256-wide psum accumulations: the
        # chain after the very last matmul is one small drain+DMA, and the
        # first quarter's drain/DMA overlaps the second quarter's matmuls
        for sub, ps_sub in ((0, ps_tr), (1, ps3)):
            acc = ps_sub[:, ds(512, 256)]
            for kt_ in range(NT):
                nc.tensor.matmul(
                    acc, sT16[:, kt_, :],
                    vals[:, kt_, ds(512 + sub * 256, 256)],
                    start=(kt_ == 0), stop=(kt_ == NT - 1),
                )
            cxq = cx_pool.tile([P, 256], F32, tag=f"cxq{sub}", bufs=1)
            if sub:
                nc.scalar.copy(cxq[:], acc)
            else:
                nc.vector.tensor_copy(cxq[:], acc)
            nc.sync.dma_start(
                c_d[b, ts(qt_, P), ds(512 + sub * 256, 256)], cxq[:])


def _phase23(nc, pools, b, s_d, c_d, qWT, kT, mrep, vals, pe_hooks=None):
    """Software-pipelined phases 2+3: step j runs p2(qt=j), the score
    transpose for qt=j-1, and the ctx matmuls for qt=j-2, keeping the PE fed
    while the ACT/DVE drains of earlier tiles complete.

    pe_hooks: optional {step: [thunk, ...]} of extra work (the next batch's
    staging load issues / qT transposes) emitted right after the p2 step."""
    s16s, sT16s = {}, {}
    MO = OPTS["mm_off"]
    TO = OPTS["sT_off"]
    for j in range(NT + MO):
        if j < NT:
            s16s[j] = _p2_step(nc, pools, b, s_d, qWT, kT, mrep, j)
        if pe_hooks:
            for thunk in pe_hooks.get(j, ()):
                thunk()
        if OPTS["sT_dma"] and TO <= j < NT + TO:
            sT16s[j - TO] = (_p3_transpose_dma(nc, pools, s16s[j - TO], j - TO), None)
        if OPTS["mm_first"] and j >= MO:
            sT16, ps_tr = sT16s[j - MO]
            _p3_mm(nc, pools, b, c_d, sT16, ps_tr, vals, j - MO)
        if not OPTS["sT_dma"] and 2 <= j <= NT + 1:
            sT16s[j - 2] = _p3_transpose(nc, pools, s16s[j - 2],
                                         dve_drain=(j >= NT + 1 - OPTS["tail_dve"]))
        if not OPTS["mm_first"] and j >= MO:
            sT16, ps_tr = sT16s[j - MO]
            _p3_mm(nc, pools, b, c_d, sT16, ps_tr, vals, j - MO,
                   last=(b == NB - 1 and j - MO == NT - 1))


def _load_values(nc, pools, b, v_d, after=None, not_before_ms=0.0):
    """SWDGE cast-load: fp32 DRAM -> fp16 SBUF, no compute engine involved.
    not_before_ms steers the static scheduler's placement; `after` (an
    earlier instruction) optionally gates the first load via a real
    dependency edge (the Pool queue is in-order, so one edge holds all
    eight)."""
    vals = pools["val"].tile([P, NT, T], F16, tag="vals")
    tc = pools["tc"]
    with tc.tile_wait_until(not_before_ms):
        for kt_ in range(NT):
            dma = nc.gpsimd.dma_start(vals[:, kt_, :], v_d[b, ts(kt_, P), :])
            if kt_ == 0 and after is not None:
                bass._add_dep_helper(dma.ins, after.ins, sync=True,
                                     reason="defer values past staging crunch")
    return vals


def _stage_mask(nc, pools, b, m_d):
    """Mask broadcast tile in fp16 (cast on the SWDGE load): costs <=5e-4
    absolute on logits of std ~32 — negligible — and halves the tile."""
    stage, small = pools["stage"], pools["small"]
    mdt = F16 if OPTS["mask_f16"] else F32
    mf = stage.tile([P, T], mdt, tag="ldmask", bufs=1)
    nc.gpsimd.dma_start(mf[:1, :], m_d[b : b + 1, :])
    mrep = small.tile([P, T], mdt, tag="mrep")
    nc.gpsimd.partition_broadcast(mrep[:], mf[:1, :])
    return mrep


def build_nc(reps=1):
    nc = bacc.Bacc("TRN2", target_bir_lowering=False, debug=False, num_devices=NCORES)
    q_d = nc.dram_tensor("query", [NB, T, T], F32, kind="ExternalInput")
    k_d = nc.dram_tensor("keys", [NB, T, T], F32, kind="ExternalInput")
    v_d = nc.dram_tensor("values", [NB, T, T], F32, kind="ExternalInput")
    w_d = nc.dram_tensor("W", [T, T], F32, kind="ExternalInput")
    m_d = nc.dram_tensor("mask", [NB, T], F32, kind="ExternalInput")
    s_d = nc.dram_tensor("score", [NB, T, T], F32, kind="ExternalOutput")
    c_d = nc.dram_tensor("ctx", [NB, T, T], F32, kind="ExternalOutput")

    with tile.TileContext(nc) as tc:
        with (
            tc.tile_pool(name="stage", bufs=2) as stage,
            tc.tile_pool(name="wres", bufs=1) as wres,
            tc.tile_pool(name="xt", bufs=1) as xt_pool,
            tc.tile_pool(name="kt", bufs=1) as kt_pool,
            tc.tile_pool(name="qwt", bufs=1) as qwt_pool,
            tc.tile_pool(name="val", bufs=1) as val_pool,
            tc.tile_pool(name="sc", bufs=1) as sc_pool,
            tc.tile_pool(name="soft", bufs=OPTS["soft_bufs"]) as soft,
            tc.tile_pool(name="st", bufs=2) as st_pool,
            tc.tile_pool(name="cx", bufs=OPTS["cx_bufs"]) as cx_pool,
            tc.tile_pool(name="small", bufs=1) as small,
            tc.tile_pool(name="ones", bufs=1) as ones_pool,
        ):
            with (
                tc.tile_pool(name="psA", bufs=2, space="PSUM") as psA,
                tc.tile_pool(name="psB", bufs=2, space="PSUM") as psB,
            ):
                pools = {
                    "tc": tc, "wres": wres,
                    "stage": stage, "xt": xt_pool, "kt": kt_pool,
                    "qwt": qwt_pool,
                    "val": val_pool, "soft": soft, "st": st_pool, "sc": sc_pool,
                    "cx": cx_pool, "small": small,
                    "psA": psA, "psB": psB,
                }
                ident16 = ones_pool.tile([P, P], F16, tag="ident16")
                from concourse.masks import make_identity
                make_identity(nc, ident16[:])
                pools["ident16"] = ident16
                identF = ones_pool.tile([P, P], F32, tag="identF")
                make_identity(nc, identF[:])
                if OPTS["ident_t"] == "f32r":
                    identR = ones_pool.tile([P, P], F32R, tag="identR")
                    # DVE, not ACT: the ACT queue is busy with the 1.3us
                    # activation-table load at kernel start, and identR
                    # gates the first (warming) PE transposes
                    nc.vector.tensor_copy(identR[:], identF[:])
                    pools["identT"] = identR
                else:
                    pools["identT"] = identF

                tdt = pools["identT"].dtype

                def mk_qdrain():
                    return lambda d, v, _i=iter(range(NT * NT)): (
                        nc.vector.tensor_copy(d, v)
                        if next(_i) >= NT - OPTS["qtail_dve"]
                        else nc.scalar.copy(d, v)
                    )

                kdrain = lambda d, v: nc.scalar.copy(d, v)
                kt_pool_name = "kt" if OPTS["split_xt"] else "xt"
                KSCHED = OPTS["kt_sched"]

                for _rep in range(reps):
                    prepared = {}
                    for b in range(NB):
                        pre_ps = None
                        if b in prepared:
                            qT, ktiles = prepared.pop(b)
                        elif OPTS["cold_fuse"] and _rep == 0 and b == 0:
                            qtiles, W_t, ktiles = _cold_stage(
                                nc, pools, q_d[b], k_d[b], w_d, tdt)
                            qT, pre_ps = _fused_qT_et01(
                                nc, pools, qtiles, W_t, mk_qdrain())
                        else:
                            cs = OPTS["cold_swdge"]
                            hp = (tc.high_priority() if OPTS["q_hiprio"]
                                  and b == 0 and _rep == 0 else None)
                            if hp is not None:
                                hp.__enter__()
                            if cs == "q0":
                                qtiles = _issue_loads(
                                    nc, pools, q_d[b], tdt, trange=[0],
                                    eng=nc.gpsimd)
                                qtiles += _issue_loads(
                                    nc, pools, q_d[b], tdt,
                                    trange=range(1, NT))
                            else:
                                qtiles = _issue_loads(
                                    nc, pools, q_d[b], tdt,
                                    eng=nc.gpsimd if "q" in cs else None)
                            if hp is not None:
                                hp.__exit__(None, None, None)
                            if _rep == 0 and b == 0:
                                W_t = _issue_w_loads(nc, pools, w_d)
                            if OPTS["warm_n0"] or OPTS["warm_n"]:
                                qT = None
                                qdr = mk_qdrain()
                                for t in range(NT):
                                    qT = _transpose_from(
                                        nc, pools, [qtiles[t]], qdr,
                                        xT=qT, t0=t,
                                        warm_first=(OPTS["warm_n0"] if t == 0
                                                    else OPTS["warm_n"]))
                            else:
                                qT = _transpose_from(nc, pools, qtiles,
                                                     mk_qdrain())
                            ktiles = _issue_loads(
                                nc, pools, k_d[b], tdt,
                                eng=nc.gpsimd if "k" in cs else None)
                        mrep = _stage_mask(nc, pools, b, m_d)
                        vals = None
                        if OPTS["v_pos"] == "early":
                            nb_ms = (OPTS["v_floor_ms"] if b == 0
                                     else (OPTS["v_floor2_ms"]
                                           or OPTS["v_floor_ms"] * (b + 1)))
                            vals = _load_values(
                                nc, pools, b, v_d, not_before_ms=nb_ms,
                            )
                        if OPTS["kT_inter"]:
                            # keys transposes ride inside the phase-1 et
                            # steps; their psum drains overlap the matmuls
                            kT = pools[kt_pool_name].tile(
                                [P, NT, T], F32R, tag="xT"
                            )
                            kt_work = {
                                et: [
                                    (lambda i=i: _transpose_from(
                                        nc, pools, [ktiles[i]], kdrain,
                                        xT=kT, t0=i))
                                    for i in idxs
                                ]
                                for et, idxs in KSCHED.items()
                            }
                            qWT, p1dr = _phase1(nc, pools, W_t, qT, kt_work,
                                                pre=pre_ps)
                        else:
                            qWT, p1dr = _phase1(nc, pools, W_t, qT, pre=pre_ps)
                            kT = _transpose_from(
                                nc, pools, ktiles, kdrain, pool=kt_pool_name,
                            )
                        if vals is None:
                            vdep = (OPTS["v_dep_et"] if b == 0
                                    else OPTS["v_dep_et_warm"])
                            vals = _load_values(
                                nc, pools, b, v_d,
                                after=p1dr[vdep] if vdep is not None else None,
                            )
                        hooks = {}
                        if OPTS["xbatch"] and b + 1 < NB:
                            nb = b + 1
                            nqtiles_box = []
                            nqT = pools["xt"].tile([P, NT, T], F32R, tag="xT")
                            nqdrain = mk_qdrain()

                            def issue_next_q(_nb=nb, _box=nqtiles_box):
                                _box.extend(_issue_loads(nc, pools, q_d[_nb], tdt))

                            hooks.setdefault(0, []).append(issue_next_q)
                            for i in range(NT):
                                hooks.setdefault(i + 1, []).append(
                                    lambda i=i, _box=nqtiles_box: _transpose_from(
                                        nc, pools, [_box[i]], nqdrain,
                                        xT=nqT, t0=i)
                                )
                            nktiles_box = []

                            def issue_next_k(_nb=nb, _box=nktiles_box):
                                _box.extend(_issue_loads(nc, pools, k_d[_nb], tdt))

                            hooks.setdefault(OPTS["xk_step"], []).append(issue_next_k)
                            prepared[nb] = (nqT, nktiles_box)
                        _phase23(nc, pools, b, s_d, c_d, qWT, kT, mrep, vals,
                                 pe_hooks=hooks or None)

    nc.compile()
    return nc


_nc = None


def _get_nc():
    global _nc
    if _nc is None:
        _nc = build_nc()
    return _nc


def make_in_maps(query, keys, values, W, mask):
    query = np.ascontiguousarray(np.asarray(query, dtype=np.float32))
    keys = np.ascontiguousarray(np.asarray(keys, dtype=np.float32))
    values = np.ascontiguousarray(np.asarray(values, dtype=np.float32))
    W = np.ascontiguousarray(np.asarray(W, dtype=np.float32))
    mask = np.ascontiguousarray(np.asarray(mask, dtype=np.float32))
    in_maps = []
    for c in range(NCORES):
        sl = slice(c * NB, (c + 1) * NB)
        in_maps.append(
            {
                "query": query[sl],
                "keys": keys[sl],
                "values": values[sl],
                "W": W,
                "mask": mask[sl],
            }
        )
    return in_maps


def kernel(query, keys, values, W, mask):
    nc = _get_nc()
    in_maps = make_in_maps(query, keys, values, W, mask)
    res = run_bass_kernel_spmd(nc, in_maps, core_ids=list(range(NCORES)))
    score = np.concatenate([res.results[c]["score"] for c in range(NCORES)], axis=0)
    ctx = np.concatenate([res.results[c]["ctx"] for c in range(NCORES)], axis=0)
    return score, ctx

